# revision 21
# baseline (speedup 1.0000x reference)
"""MultiHeadDiffAttn Trainium2 kernel (v2, 16-bit matmul path).

Sharding: 8 cores = 4-way data parallel over batch x 2-way tensor parallel
over heads (8 v-heads / 16 half-heads per core).  Each core computes its
batch's qkv projection restricted to its head group, differential attention
with per-half-head softmax, head RMS norm, and a partial output projection
(its 512 rows of w_proj).  Host sums the two partial projections per batch.

Measured device behavior this kernel is shaped around:
  - fp32 matmul streams at ~1/4 the 16-bit rate (and fp32 K=32 at 1/8), so
    all matmul operands are fp16 (PSUM accumulation stays fp32).
  - K=32 16-bit matmuls stream at 2 cycles/col; the two half-heads of a
    head sit at different 32-row strips of the PE array, so their S^T
    matmuls are emitted adjacently with explicit tile_position to overlap.
  - ACT exp costs ~0.83ns/col + ~360ns/op fixed, so exp runs once per
    (half-head, s-block) over the whole [128, up-to-1024] PSUM row-block.
  - AV runs transposed (U^T[dv,t] = v_aug^T @ expS^T) so it is
    stream-bound with a tiny reused LDWEIGHTS; a ones-column appended to v
    makes row 64 of U^T the softmax denominator.  The den row is
    reciprocal'd in SBUF before PE-transposing U^T back to [t, dv+1], so
    normalization + lambda-combine + RMS are native per-partition ops.
"""

import math
from contextlib import ExitStack

import numpy as np

import concourse.bass as bass
import concourse.tile as tile
from concourse import masks, mybir
from concourse.bass_utils import run_bass_kernel_spmd

# The deployed walrus rejects instructions carrying more than one sync wait
# ("Too many sync wait commands" in setupSyncWait).  Legalize at the BIR-JSON
# level: for every instruction with >1 wait, hoist the extra waits onto NoOp
# instructions inserted just before it on the same engine (engine streams are
# in-order, so semantics are identical).
_MAX_WAITS = 1


def _legalize_sync_waits(d):
    for f in d.get("functions", []):
        for bb in f.get("blocks", []):
            out = []
            for inst in bb["instructions"]:
                si = inst.get("sync_info")
                waits = (si or {}).get("on_wait") or []
                if len(waits) > _MAX_WAITS:
                    extra = waits[: len(waits) - _MAX_WAITS]
                    keep = waits[len(waits) - _MAX_WAITS :]
                    for j in range(0, len(extra), _MAX_WAITS):
                        nop = {
                            "engine": inst["engine"],
                            "ins": [],
                            "outs": [],
                            "name": f"{inst['name']}-lw{j}",
                            "opcode": "NoOp",
                            "sync_info": {
                                "on_wait": extra[j : j + _MAX_WAITS],
                                "on_update": [],
                            },
                        }
                        if "debug" in inst:
                            nop["debug"] = inst["debug"]
                        out.append(nop)
                    si["on_wait"] = keep
                out.append(inst)
            bb["instructions"] = out
    return d


_orig_to_json_bytes = bass.Bass.to_json_bytes


def _patched_to_json_bytes(self, *a, **kw):
    import json as _json

    raw = _orig_to_json_bytes(self, *a, **kw)
    return _json.dumps(_legalize_sync_waits(_json.loads(raw))).encode()


bass.Bass.to_json_bytes = _patched_to_json_bytes

F32 = mybir.dt.float32
F16 = mybir.dt.float16

B, T, C = 4, 1024, 1024
H_TOT = 16  # total v-heads
HD = 32  # half-head dim
DV = 64  # v-head dim
G = 2  # head groups (tensor parallel)
HPG = H_TOT // G  # 8 v-heads per core
COLS = 1024  # q cols + k cols per group
LAMBDA_INIT = 0.8 - 0.6 * math.exp(-0.3 * (1 - 1))  # 0.2
EPS = 1e-5
N_CORES = 8

NT = T // 128  # 8 t-tiles
NKC = C // 128  # 8 contraction chunks


def _emit(ctx: ExitStack, tc: tile.TileContext, xT, w_qk, w_v, w_p, lam, y):
    nc = tc.nc
    AluOp = mybir.AluOpType
    Act = mybir.ActivationFunctionType

    const = ctx.enter_context(tc.tile_pool(name="const", bufs=1))
    ident = const.tile([128, 128], F16)
    masks.make_identity(nc, ident[:])
    lam_sb = const.tile([128, 1], F32)
    nc.sync.dma_start(out=lam_sb, in_=lam[:])
    eps_sb = const.tile([128, 1], F32)
    nc.vector.memset(eps_sb, EPS)

    big = ctx.enter_context(tc.tile_pool(name="big", bufs=1))
    qkT_sb = big.tile([128, 8, T], F16)  # row-chunks of [COLS, T]
    v_sb = big.tile([128, NT, HPG, 128], F16)  # [s-chunk][head][dv | ones | 0-pad]
    outcat_sb = big.tile([128, NT, HPG * DV], F16)  # [t-chunk][512]
    outcatT_sb = big.tile([128, 4, T], F16)  # row-chunks of [512, T]
    wp_sb = big.tile([128, 4, C], F16)
    # per-half-head q, zero-padded to K=128: data lives at the same 32-row
    # strip as that half-head's k rows inside its qkT chunk, so the S^T
    # matmul can contract over the full 128 partitions at full stream rate
    # (the other half-heads' k rows meet zero q rows).
    qTp_sb = big.tile([128, 2 * HPG, T], F16)

    nc.gpsimd.memset(v_sb[:, :, :, DV + 1 :], 0.0)
    nc.gpsimd.memset(qTp_sb, 0.0)

    # ---------------- phase 1+2: qkv projections ----------------
    with (
        tc.tile_pool(name="xw", bufs=1) as xw,
        tc.tile_pool(name="mmps", bufs=4, space="PSUM") as mmps,
    ):
        xT_sb = xw.tile([128, NKC, T], F16)
        wqk_sb = xw.tile([128, NKC, COLS], F16)
        wv_sb = xw.tile([128, NKC, 512], F16)

        xT_r = xT[:].rearrange("(c p) t -> p c t", p=128)
        wqk_r = w_qk[:].rearrange("(c p) m -> p c m", p=128)

        def load_wqk(cc):
            nc.sync.dma_start(
                out=wqk_sb[:, :, cc * 128 : (cc + 1) * 128],
                in_=wqk_r[:, :, cc * 128 : (cc + 1) * 128],
            )

        load_wqk(0)
        for nh in range(2):  # t-halves so first matmuls start early
            nc.sync.dma_start(
                out=xT_sb[:, :, nh * 512 : (nh + 1) * 512],
                in_=xT_r[:, :, nh * 512 : (nh + 1) * 512],
            )
        for cc in range(1, 8):
            load_wqk(cc)
        nc.sync.dma_start(out=wv_sb, in_=w_v[:].rearrange("(c p) m -> p c m", p=128))
        nc.sync.dma_start(
            out=wp_sb, in_=w_p[:].rearrange("(c p) m -> p c m", p=128)
        )

        # qkT[cc-block, :] = w_qk[:, cc-block].T @ x^T
        for cc in range(8):
            for nh in range(2):
                ps = mmps.tile([128, 512], F32, tag="mmps", name=f"qk{cc}{nh}")
                for kc in range(NKC):
                    nc.tensor.matmul(
                        ps,
                        wqk_sb[:, kc, cc * 128 : (cc + 1) * 128],
                        xT_sb[:, kc, nh * 512 : (nh + 1) * 512],
                        start=(kc == 0),
                        stop=(kc == NKC - 1),
                    )
                nc.vector.tensor_copy(
                    out=qkT_sb[:, cc, nh * 512 : (nh + 1) * 512], in_=ps
                )
                if cc < 4:  # q chunk: scatter the 4 half-heads into qTp
                    for j in range(4):
                        hh = cc * 4 + j
                        nc.vector.tensor_copy(
                            out=qTp_sb[
                                j * 32 : (j + 1) * 32,
                                hh,
                                nh * 512 : (nh + 1) * 512,
                            ],
                            in_=qkT_sb[
                                j * 32 : (j + 1) * 32,
                                cc,
                                nh * 512 : (nh + 1) * 512,
                            ],
                        )

        # v[t-block, :] = x @ w_v ; scatter heads into v_sb, slot 64 = ones
        for tt in range(NT):
            ps = mmps.tile([128, 512], F32, tag="mmps", name=f"v{tt}")
            for kc in range(NKC):
                nc.tensor.matmul(
                    ps,
                    xT_sb[:, kc, tt * 128 : (tt + 1) * 128],
                    wv_sb[:, kc, :],
                    start=(kc == 0),
                    stop=(kc == NKC - 1),
                )
            nc.vector.tensor_copy(
                out=v_sb[:, tt, :, 0:DV],
                in_=ps[:].rearrange("p (h d) -> p h d", h=HPG),
            )
            nc.vector.memset(v_sb[:, tt, :, DV : DV + 1], 1.0)

    # ---------------- phase 3: differential attention ----------------
    with (
        tc.tile_pool(name="es", bufs=3) as es_pool,
        tc.tile_pool(name="us", bufs=2) as us_pool,
        tc.tile_pool(name="sps", bufs=1, space="PSUM") as s_pool,
        tc.tile_pool(name="ups", bufs=1, space="PSUM") as u_pool,
        tc.tile_pool(name="comb", bufs=3) as comb,
        tc.tile_pool(name="ohp", bufs=1, space="SBUF") as ohp,
    ):
        oh_all = {}
        ssq_all = ohp.tile([128, HPG * NT], F32, tag="ssqall", name="ssqall")

        def emit_av(h, s, e, es_s, chunks):
            for c0, c1 in chunks:
                nc.tensor.matmul(
                    u_tiles[e][:, c0:c1],
                    v_sb[:, s, h, :],
                    es_s[:, c0:c1],
                    start=(s == 0),
                    stop=(s == 3 and c1 == 512) or (s == 7),
                )

        for h in range(HPG):
            qc = h // 2
            kc_ = 4 + h // 2
            pbase = [(2 * h % 4) * 32, (2 * h % 4) * 32 + 32]
            s_comb = s_pool.tile([128, 2, T], F32, tag="scomb", name=f"sc_{h}")
            u_tiles = [
                u_pool.tile([128, T], F32, tag=f"u{e}", name=f"u{e}_{h}")
                for e in range(2)
            ]
            prev = None  # both AVs delayed one s-iteration
            for s in range(NT):
                t0 = 128 * s
                chunks = [(t0, 512), (512, 1024)] if s < 4 else [(t0, 1024)]
                es_s = es_pool.tile([128, 2, T], F16, tag="es", name=f"es_{h}_{s}")
                for c0, c1 in chunks:
                    for e in range(2):
                        nc.tensor.matmul(
                            s_comb[:, e, c0:c1],
                            qkT_sb[:, kc_, t0 : t0 + 128],
                            qTp_sb[:, 2 * h + e, c0:c1],
                            start=True,
                            stop=True,
                        )
                if prev is not None:
                    ps_, pes_, pchunks_ = prev
                    emit_av(h, ps_, 0, pes_[:, 0, :], pchunks_)
                    emit_av(h, ps_, 1, pes_[:, 1, :], pchunks_)
                # one exp covers both half-heads' row-blocks (2-segment AP)
                nc.scalar.activation(
                    out=es_s[:, :, t0:T],
                    in_=s_comb[:, :, t0:T],
                    func=Act.Exp,
                    scale=1.0 / 32.0,
                )
                for e in range(2):
                    # causal mask inside the diagonal block: keep t >= s
                    nc.gpsimd.affine_select(
                        out=es_s[:, e, t0 : t0 + 128],
                        in_=es_s[:, e, t0 : t0 + 128],
                        pattern=[[1, 128]],
                        compare_op=AluOp.is_ge,
                        fill=0.0,
                        base=0,
                        channel_multiplier=-1,
                    )
                prev = (s, es_s, chunks)
            ps_, pes_, pchunks_ = prev
            emit_av(h, ps_, 0, pes_[:, 0, :], pchunks_)
            emit_av(h, ps_, 1, pes_[:, 1, :], pchunks_)

            # ---- copy U^T to SBUF (f16), DMA-transpose to [t, dv|den] ----
            us_tiles = [
                us_pool.tile([128, T], F16, tag=f"us{e}", name=f"us{e}_{h}")
                for e in range(2)
            ]
            tsb_tiles = [
                us_pool.tile([128, NT, 128], F16, tag=f"tsb{e}", name=f"tsb{e}_{h}")
                for e in range(2)
            ]
            for e in range(2):
                nc.vector.tensor_copy(out=us_tiles[e], in_=u_tiles[e])
                nc.sync.dma_start_transpose(out=tsb_tiles[e], in_=us_tiles[e])

            # ---- normalize, lambda-combine, accumulate sum-of-squares ----
            for tj in range(NT):
                rr_ = [
                    comb.tile([128, 1], F32, tag=f"r{e}", name=f"r{e}_{h}_{tj}")
                    for e in range(2)
                ]
                for e in range(2):
                    nc.vector.reciprocal(
                        out=rr_[e], in_=tsb_tiles[e][:, tj, DV : DV + 1]
                    )
                t2 = comb.tile([128, DV], F32, tag="t2", name=f"t2_{h}_{tj}")
                nc.vector.tensor_scalar(
                    out=t2,
                    in0=tsb_tiles[1][:, tj, 0:DV],
                    scalar1=rr_[1][:],
                    scalar2=lam_sb[:],
                    op0=AluOp.mult,
                    op1=AluOp.mult,
                )
                oh = ohp.tile(
                    [128, DV], F32, tag=f"oh{h}_{tj}", name=f"oh_{h}_{tj}"
                )
                nc.vector.scalar_tensor_tensor(
                    out=oh,
                    in0=tsb_tiles[0][:, tj, 0:DV],
                    scalar=rr_[0][:],
                    in1=t2,
                    op0=AluOp.mult,
                    op1=AluOp.subtract,
                )
                sq = comb.tile([128, DV], F32, tag="sq", name=f"sq_{h}_{tj}")
                nc.vector.scalar_tensor_tensor(
                    out=sq,
                    in0=oh,
                    scalar=1.0,
                    in1=oh,
                    op0=AluOp.mult,
                    op1=AluOp.mult,
                    accum_out=ssq_all[:, h * NT + tj : h * NT + tj + 1],
                )
                oh_all[(h, tj)] = oh

        # ---- deferred RMS: one batched Sqrt for all heads ----
        rstd = comb.tile([128, HPG * NT], F32, tag="rstd", name="rstd")
        nc.scalar.activation(
            out=rstd, in_=ssq_all, func=Act.Sqrt, bias=eps_sb[:], scale=1.0 / DV
        )
        nc.vector.reciprocal(out=rstd, in_=rstd)
        for h in range(HPG):
            for tj in range(NT):
                nc.vector.tensor_scalar_mul(
                    out=outcat_sb[:, tj, h * DV : (h + 1) * DV],
                    in0=oh_all[(h, tj)],
                    scalar1=rstd[:, h * NT + tj : h * NT + tj + 1],
                )

    # ---------------- phase 4+5: transpose + output projection ----------------
    with (
        tc.tile_pool(name="tps", bufs=2, space="PSUM") as tps,
        tc.tile_pool(name="pps", bufs=4, space="PSUM") as pps,
        tc.tile_pool(name="yout", bufs=2) as yout,
    ):
        for rr in range(4):
            for tt in range(NT):
                ps = tps.tile([128, 128], F16, tag="tp", name=f"ot{rr}{tt}")
                nc.tensor.transpose(
                    ps, outcat_sb[:, tt, rr * 128 : (rr + 1) * 128], ident
                )
                nc.vector.tensor_copy(
                    out=outcatT_sb[:, rr, tt * 128 : (tt + 1) * 128], in_=ps
                )
        for tt in range(NT):
            yt = yout.tile([128, C], F32, tag="yt", name=f"y{tt}")
            for nh in range(2):
                ps = pps.tile([128, 512], F32, tag="pp", name=f"pp{tt}{nh}")
                for rr in range(4):
                    nc.tensor.matmul(
                        ps,
                        outcatT_sb[:, rr, tt * 128 : (tt + 1) * 128],
                        wp_sb[:, rr, nh * 512 : (nh + 1) * 512],
                        start=(rr == 0),
                        stop=(rr == 3),
                    )
                nc.vector.tensor_copy(out=yt[:, nh * 512 : (nh + 1) * 512], in_=ps)
            nc.sync.dma_start(out=y[tt * 128 : (tt + 1) * 128, :], in_=yt)


def build_nc():
    nc = bass.Bass()
    xT = nc.declare_dram_parameter("xT", [C, T], F16, isOutput=False)
    w_qk = nc.declare_dram_parameter("w_qk", [C, COLS], F16, isOutput=False)
    w_v = nc.declare_dram_parameter("w_v", [C, 512], F16, isOutput=False)
    w_p = nc.declare_dram_parameter("w_p", [512, C], F16, isOutput=False)
    lam = nc.declare_dram_parameter("lam", [128, 1], F32, isOutput=False)
    y = nc.declare_dram_parameter("y", [T, C], F32, isOutput=True)
    with tile.TileContext(nc) as tc:
        with ExitStack() as ctx:
            _emit(ctx, tc, xT, w_qk, w_v, w_p, lam, y)
    return nc


_NC = None


def _get_nc():
    global _NC
    if _NC is None:
        _NC = build_nc()
    return _NC


def make_in_maps(x, w_attn, w_proj, lambda_q1, lambda_q2, lambda_k1, lambda_k2, gamma):
    x = np.asarray(x, np.float32)
    w_attn = np.asarray(w_attn, np.float32)
    w_proj = np.asarray(w_proj, np.float32)
    lam1 = np.exp(np.sum(np.float32(lambda_q1) * np.float32(lambda_k1), dtype=np.float32))
    lam2 = np.exp(np.sum(np.float32(lambda_q2) * np.float32(lambda_k2), dtype=np.float32))
    lam_full = np.float32(lam1 - lam2 + LAMBDA_INIT)
    lam_tile = np.full((128, 1), lam_full, np.float32)
    # fold gamma * (1 - lambda_init) into w_proj rows
    scale = np.tile(np.asarray(gamma, np.float32), H_TOT) * np.float32(1.0 - LAMBDA_INIT)
    w_p_full = (w_proj * scale[:, None]).astype(np.float16)

    in_maps = []
    for core in range(N_CORES):
        b, g = core // G, core % G
        in_maps.append(
            {
                "xT": np.ascontiguousarray(x[b].T.astype(np.float16)),
                "w_qk": np.ascontiguousarray(
                    np.concatenate(
                        [
                            w_attn[:, g * 512 : (g + 1) * 512],
                            w_attn[:, C + g * 512 : C + (g + 1) * 512],
                        ],
                        axis=1,
                    ).astype(np.float16)
                ),
                "w_v": np.ascontiguousarray(
                    w_attn[:, 2 * C + g * 512 : 2 * C + (g + 1) * 512].astype(
                        np.float16
                    )
                ),
                "w_p": np.ascontiguousarray(w_p_full[g * 512 : (g + 1) * 512, :]),
                "lam": lam_tile,
            }
        )
    return in_maps


def assemble(results):
    y = np.empty((B, T, C), np.float32)
    for b in range(B):
        y[b] = results[b * G]["y"] + results[b * G + 1]["y"]
    return y


def kernel(**inputs) -> np.ndarray:
    nc = _get_nc()
    in_maps = make_in_maps(**inputs)
    res = run_bass_kernel_spmd(nc, in_maps, list(range(N_CORES)))
    return assemble(res.results)


# revision 22
# speedup vs baseline: 1.4036x; 1.4036x over previous
"""MultiHeadDiffAttn Trainium2 kernel (v2, 16-bit matmul path).

Sharding: 8 cores = 4-way data parallel over batch x 2-way tensor parallel
over heads (8 v-heads / 16 half-heads per core).  Each core computes its
batch's qkv projection restricted to its head group, differential attention
with per-half-head softmax, head RMS norm, and a partial output projection
(its 512 rows of w_proj).  Host sums the two partial projections per batch.

Measured device behavior this kernel is shaped around:
  - fp32 matmul streams at ~1/4 the 16-bit rate (and fp32 K=32 at 1/8), so
    all matmul operands are fp16 (PSUM accumulation stays fp32).
  - K=32 16-bit matmuls stream at 2 cycles/col; the two half-heads of a
    head sit at different 32-row strips of the PE array, so their S^T
    matmuls are emitted adjacently with explicit tile_position to overlap.
  - ACT exp costs ~0.83ns/col + ~360ns/op fixed, so exp runs once per
    (half-head, s-block) over the whole [128, up-to-1024] PSUM row-block.
  - AV runs transposed (U^T[dv,t] = v_aug^T @ expS^T) so it is
    stream-bound with a tiny reused LDWEIGHTS; a ones-column appended to v
    makes row 64 of U^T the softmax denominator.  The den row is
    reciprocal'd in SBUF before PE-transposing U^T back to [t, dv+1], so
    normalization + lambda-combine + RMS are native per-partition ops.
"""

import math
from contextlib import ExitStack

import numpy as np

import concourse.bass as bass
import concourse.tile as tile
from concourse import masks, mybir
from concourse.bass_utils import run_bass_kernel_spmd

# The deployed walrus rejects instructions carrying more than one sync wait
# ("Too many sync wait commands" in setupSyncWait).  Legalize at the BIR-JSON
# level: for every instruction with >1 wait, hoist the extra waits onto NoOp
# instructions inserted just before it on the same engine (engine streams are
# in-order, so semantics are identical).
_MAX_WAITS = 1


def _legalize_sync_waits(d):
    for f in d.get("functions", []):
        for bb in f.get("blocks", []):
            out = []
            for inst in bb["instructions"]:
                si = inst.get("sync_info")
                waits = (si or {}).get("on_wait") or []
                if len(waits) > _MAX_WAITS:
                    extra = waits[: len(waits) - _MAX_WAITS]
                    keep = waits[len(waits) - _MAX_WAITS :]
                    for j in range(0, len(extra), _MAX_WAITS):
                        nop = {
                            "engine": inst["engine"],
                            "ins": [],
                            "outs": [],
                            "name": f"{inst['name']}-lw{j}",
                            "opcode": "NoOp",
                            "sync_info": {
                                "on_wait": extra[j : j + _MAX_WAITS],
                                "on_update": [],
                            },
                        }
                        if "debug" in inst:
                            nop["debug"] = inst["debug"]
                        out.append(nop)
                    si["on_wait"] = keep
                out.append(inst)
            bb["instructions"] = out
    return d


_orig_to_json_bytes = bass.Bass.to_json_bytes


def _patched_to_json_bytes(self, *a, **kw):
    import json as _json

    raw = _orig_to_json_bytes(self, *a, **kw)
    return _json.dumps(_legalize_sync_waits(_json.loads(raw))).encode()


bass.Bass.to_json_bytes = _patched_to_json_bytes

F32 = mybir.dt.float32
F16 = mybir.dt.float16

B, T, C = 4, 1024, 1024
H_TOT = 16  # total v-heads
HD = 32  # half-head dim
DV = 64  # v-head dim
G = 2  # head groups (tensor parallel)
HPG = H_TOT // G  # 8 v-heads per core
COLS = 1024  # q cols + k cols per group
LAMBDA_INIT = 0.8 - 0.6 * math.exp(-0.3 * (1 - 1))  # 0.2
EPS = 1e-5
N_CORES = 8

NT = T // 128  # 8 t-tiles
NKC = C // 128  # 8 contraction chunks


def _emit(ctx: ExitStack, tc: tile.TileContext, xT, w_qk, w_v, w_p, lam, y):
    nc = tc.nc
    AluOp = mybir.AluOpType
    Act = mybir.ActivationFunctionType

    const = ctx.enter_context(tc.tile_pool(name="const", bufs=1))
    ident = const.tile([128, 128], F16)
    masks.make_identity(nc, ident[:])
    lam_sb = const.tile([128, 1], F32)
    nc.sync.dma_start(out=lam_sb, in_=lam[:])
    eps_sb = const.tile([128, 1], F32)
    nc.vector.memset(eps_sb, EPS)

    big = ctx.enter_context(tc.tile_pool(name="big", bufs=1))
    qkT_sb = big.tile([128, 8, T], F16)  # row-chunks of [COLS, T]
    v_sb = big.tile([128, NT, HPG, 128], F16)  # [s-chunk][head][dv | ones | 0-pad]
    outcat_sb = big.tile([128, NT, HPG * DV], F16)  # [t-chunk][512]
    outcatT_sb = big.tile([128, 4, T], F16)  # row-chunks of [512, T]
    wp_sb = big.tile([128, 4, C], F16)
    # per-half-head q, zero-padded to K=128: data lives at the same 32-row
    # strip as that half-head's k rows inside its qkT chunk, so the S^T
    # matmul can contract over the full 128 partitions at full stream rate
    # (the other half-heads' k rows meet zero q rows).
    qTp_sb = big.tile([128, 2 * HPG, T], F16)

    nc.gpsimd.memset(v_sb[:, :, :, DV + 1 :], 0.0)
    nc.gpsimd.memset(qTp_sb, 0.0)

    # ---------------- phase 1+2: qkv projections ----------------
    with (
        tc.tile_pool(name="xw", bufs=1) as xw,
        tc.tile_pool(name="mmps", bufs=4, space="PSUM") as mmps,
    ):
        xT_sb = xw.tile([128, NKC, T], F16)
        wqk_sb = xw.tile([128, NKC, COLS], F16)
        wv_sb = xw.tile([128, NKC, 512], F16)

        xT_r = xT[:].rearrange("(c p) t -> p c t", p=128)
        wqk_r = w_qk[:].rearrange("(c p) m -> p c m", p=128)

        def load_wqk(cc):
            nc.sync.dma_start(
                out=wqk_sb[:, :, cc * 128 : (cc + 1) * 128],
                in_=wqk_r[:, :, cc * 128 : (cc + 1) * 128],
            )

        load_wqk(0)
        for nh in range(2):  # t-halves so first matmuls start early
            nc.sync.dma_start(
                out=xT_sb[:, :, nh * 512 : (nh + 1) * 512],
                in_=xT_r[:, :, nh * 512 : (nh + 1) * 512],
            )
        for cc in range(1, 8):
            load_wqk(cc)
        nc.sync.dma_start(out=wv_sb, in_=w_v[:].rearrange("(c p) m -> p c m", p=128))
        nc.sync.dma_start(
            out=wp_sb, in_=w_p[:].rearrange("(c p) m -> p c m", p=128)
        )

        # qkT[cc-block, :] = w_qk[:, cc-block].T @ x^T
        for cc in range(8):
            for nh in range(2):
                ps = mmps.tile([128, 512], F32, tag="mmps", name=f"qk{cc}{nh}")
                for kc in range(NKC):
                    nc.tensor.matmul(
                        ps,
                        wqk_sb[:, kc, cc * 128 : (cc + 1) * 128],
                        xT_sb[:, kc, nh * 512 : (nh + 1) * 512],
                        start=(kc == 0),
                        stop=(kc == NKC - 1),
                    )
                nc.vector.tensor_copy(
                    out=qkT_sb[:, cc, nh * 512 : (nh + 1) * 512], in_=ps
                )
                if cc < 4:  # q chunk: scatter the 4 half-heads into qTp
                    for j in range(4):
                        hh = cc * 4 + j
                        nc.vector.tensor_copy(
                            out=qTp_sb[
                                j * 32 : (j + 1) * 32,
                                hh,
                                nh * 512 : (nh + 1) * 512,
                            ],
                            in_=qkT_sb[
                                j * 32 : (j + 1) * 32,
                                cc,
                                nh * 512 : (nh + 1) * 512,
                            ],
                        )

        # v[t-block, :] = x @ w_v ; scatter heads into v_sb, slot 64 = ones
        for tt in range(NT):
            ps = mmps.tile([128, 512], F32, tag="mmps", name=f"v{tt}")
            for kc in range(NKC):
                nc.tensor.matmul(
                    ps,
                    xT_sb[:, kc, tt * 128 : (tt + 1) * 128],
                    wv_sb[:, kc, :],
                    start=(kc == 0),
                    stop=(kc == NKC - 1),
                )
            nc.vector.tensor_copy(
                out=v_sb[:, tt, :, 0:DV],
                in_=ps[:].rearrange("p (h d) -> p h d", h=HPG),
            )
            nc.vector.memset(v_sb[:, tt, :, DV : DV + 1], 1.0)

    # ---------------- phase 3: differential attention ----------------
    with (
        tc.tile_pool(name="es", bufs=3) as es_pool,
        tc.tile_pool(name="us", bufs=2) as us_pool,
        tc.tile_pool(name="sps", bufs=1, space="PSUM") as s_pool,
        tc.tile_pool(name="ups", bufs=1, space="PSUM") as u_pool,
        tc.tile_pool(name="comb", bufs=3) as comb,
        tc.tile_pool(name="ohp", bufs=1, space="SBUF") as ohp,
    ):
        oh_all = {}
        ssq_all = ohp.tile([128, HPG * NT], F32, tag="ssqall", name="ssqall")

        def emit_av(h, s, e, es_s, chunks):
            for c0, c1 in chunks:
                nc.tensor.matmul(
                    u_tiles[e][:, c0:c1],
                    v_sb[:, s, h, :],
                    es_s[:, c0:c1],
                    start=(s == 0),
                    stop=(s == 3 and c1 == 512) or (s == 7),
                )

        for h in range(HPG):
            qc = h // 2
            kc_ = 4 + h // 2
            pbase = [(2 * h % 4) * 32, (2 * h % 4) * 32 + 32]
            s_tiles = [
                s_pool.tile([128, T], F32, tag=f"s{e}", name=f"s{e}_{h}")
                for e in range(2)
            ]
            u_tiles = [
                u_pool.tile([128, T], F32, tag=f"u{e}", name=f"u{e}_{h}")
                for e in range(2)
            ]
            prev = None  # both AVs delayed one s-iteration
            for s in range(NT):
                t0 = 128 * s
                chunks = [(t0, 512), (512, 1024)] if s < 4 else [(t0, 1024)]
                es_s = [
                    es_pool.tile([128, T], F16, tag=f"es{e}", name=f"es{e}_{h}_{s}")
                    for e in range(2)
                ]
                for c0, c1 in chunks:
                    for e in range(2):
                        nc.tensor.matmul(
                            s_tiles[e][:, c0:c1],
                            qkT_sb[:, kc_, t0 : t0 + 128],
                            qTp_sb[:, 2 * h + e, c0:c1],
                            start=True,
                            stop=True,
                        )
                if prev is not None:
                    ps_, pes_, pchunks_ = prev
                    emit_av(h, ps_, 0, pes_[0], pchunks_)
                    emit_av(h, ps_, 1, pes_[1], pchunks_)
                for e in range(2):
                    nc.scalar.activation(
                        out=es_s[e][:, t0:T],
                        in_=s_tiles[e][:, t0:T],
                        func=Act.Exp,
                        scale=1.0 / 32.0,
                    )
                    # causal mask inside the diagonal block: keep t >= s
                    nc.gpsimd.affine_select(
                        out=es_s[e][:, t0 : t0 + 128],
                        in_=es_s[e][:, t0 : t0 + 128],
                        pattern=[[1, 128]],
                        compare_op=AluOp.is_ge,
                        fill=0.0,
                        base=0,
                        channel_multiplier=-1,
                    )
                prev = (s, es_s, chunks)
            ps_, pes_, pchunks_ = prev
            emit_av(h, ps_, 0, pes_[0], pchunks_)
            emit_av(h, ps_, 1, pes_[1], pchunks_)

            # ---- copy U^T to SBUF (f16), DMA-transpose to [t, dv|den] ----
            us_tiles = [
                us_pool.tile([128, T], F16, tag=f"us{e}", name=f"us{e}_{h}")
                for e in range(2)
            ]
            tsb_tiles = [
                us_pool.tile([128, NT, 128], F16, tag=f"tsb{e}", name=f"tsb{e}_{h}")
                for e in range(2)
            ]
            for e in range(2):
                nc.vector.tensor_copy(out=us_tiles[e], in_=u_tiles[e])
                nc.sync.dma_start_transpose(out=tsb_tiles[e], in_=us_tiles[e])

            # ---- normalize, lambda-combine, accumulate sum-of-squares ----
            for tj in range(NT):
                rr_ = [
                    comb.tile([128, 1], F32, tag=f"r{e}", name=f"r{e}_{h}_{tj}")
                    for e in range(2)
                ]
                for e in range(2):
                    nc.vector.reciprocal(
                        out=rr_[e], in_=tsb_tiles[e][:, tj, DV : DV + 1]
                    )
                t2 = comb.tile([128, DV], F32, tag="t2", name=f"t2_{h}_{tj}")
                nc.vector.tensor_scalar(
                    out=t2,
                    in0=tsb_tiles[1][:, tj, 0:DV],
                    scalar1=rr_[1][:],
                    scalar2=lam_sb[:],
                    op0=AluOp.mult,
                    op1=AluOp.mult,
                )
                oh = ohp.tile(
                    [128, DV], F32, tag=f"oh{h}_{tj}", name=f"oh_{h}_{tj}"
                )
                nc.vector.scalar_tensor_tensor(
                    out=oh,
                    in0=tsb_tiles[0][:, tj, 0:DV],
                    scalar=rr_[0][:],
                    in1=t2,
                    op0=AluOp.mult,
                    op1=AluOp.subtract,
                )
                sq = comb.tile([128, DV], F32, tag="sq", name=f"sq_{h}_{tj}")
                nc.vector.scalar_tensor_tensor(
                    out=sq,
                    in0=oh,
                    scalar=1.0,
                    in1=oh,
                    op0=AluOp.mult,
                    op1=AluOp.mult,
                    accum_out=ssq_all[:, h * NT + tj : h * NT + tj + 1],
                )
                oh_all[(h, tj)] = oh

        # ---- deferred RMS: one batched Sqrt for all heads ----
        rstd = comb.tile([128, HPG * NT], F32, tag="rstd", name="rstd")
        nc.scalar.activation(
            out=rstd, in_=ssq_all, func=Act.Sqrt, bias=eps_sb[:], scale=1.0 / DV
        )
        nc.vector.reciprocal(out=rstd, in_=rstd)
        for h in range(HPG):
            for tj in range(NT):
                nc.vector.tensor_scalar_mul(
                    out=outcat_sb[:, tj, h * DV : (h + 1) * DV],
                    in0=oh_all[(h, tj)],
                    scalar1=rstd[:, h * NT + tj : h * NT + tj + 1],
                )

    # ---------------- phase 4+5: transpose + output projection ----------------
    with (
        tc.tile_pool(name="tps", bufs=2, space="PSUM") as tps,
        tc.tile_pool(name="pps", bufs=4, space="PSUM") as pps,
        tc.tile_pool(name="yout", bufs=2) as yout,
    ):
        for rr in range(4):
            for tt in range(NT):
                ps = tps.tile([128, 128], F16, tag="tp", name=f"ot{rr}{tt}")
                nc.tensor.transpose(
                    ps, outcat_sb[:, tt, rr * 128 : (rr + 1) * 128], ident
                )
                nc.vector.tensor_copy(
                    out=outcatT_sb[:, rr, tt * 128 : (tt + 1) * 128], in_=ps
                )
        for tt in range(NT):
            yt = yout.tile([128, C], F32, tag="yt", name=f"y{tt}")
            for nh in range(2):
                ps = pps.tile([128, 512], F32, tag="pp", name=f"pp{tt}{nh}")
                for rr in range(4):
                    nc.tensor.matmul(
                        ps,
                        outcatT_sb[:, rr, tt * 128 : (tt + 1) * 128],
                        wp_sb[:, rr, nh * 512 : (nh + 1) * 512],
                        start=(rr == 0),
                        stop=(rr == 3),
                    )
                nc.vector.tensor_copy(out=yt[:, nh * 512 : (nh + 1) * 512], in_=ps)
            nc.sync.dma_start(out=y[tt * 128 : (tt + 1) * 128, :], in_=yt)


def build_nc():
    nc = bass.Bass()
    xT = nc.declare_dram_parameter("xT", [C, T], F16, isOutput=False)
    w_qk = nc.declare_dram_parameter("w_qk", [C, COLS], F16, isOutput=False)
    w_v = nc.declare_dram_parameter("w_v", [C, 512], F16, isOutput=False)
    w_p = nc.declare_dram_parameter("w_p", [512, C], F16, isOutput=False)
    lam = nc.declare_dram_parameter("lam", [128, 1], F32, isOutput=False)
    y = nc.declare_dram_parameter("y", [T, C], F32, isOutput=True)
    with tile.TileContext(nc) as tc:
        with ExitStack() as ctx:
            _emit(ctx, tc, xT, w_qk, w_v, w_p, lam, y)
    return nc


_NC = None


def _get_nc():
    global _NC
    if _NC is None:
        _NC = build_nc()
    return _NC


def make_in_maps(x, w_attn, w_proj, lambda_q1, lambda_q2, lambda_k1, lambda_k2, gamma):
    x = np.asarray(x, np.float32)
    w_attn = np.asarray(w_attn, np.float32)
    w_proj = np.asarray(w_proj, np.float32)
    lam1 = np.exp(np.sum(np.float32(lambda_q1) * np.float32(lambda_k1), dtype=np.float32))
    lam2 = np.exp(np.sum(np.float32(lambda_q2) * np.float32(lambda_k2), dtype=np.float32))
    lam_full = np.float32(lam1 - lam2 + LAMBDA_INIT)
    lam_tile = np.full((128, 1), lam_full, np.float32)
    # fold gamma * (1 - lambda_init) into w_proj rows
    scale = np.tile(np.asarray(gamma, np.float32), H_TOT) * np.float32(1.0 - LAMBDA_INIT)
    w_p_full = (w_proj * scale[:, None]).astype(np.float16)

    in_maps = []
    for core in range(N_CORES):
        b, g = core // G, core % G
        in_maps.append(
            {
                "xT": np.ascontiguousarray(x[b].T.astype(np.float16)),
                "w_qk": np.ascontiguousarray(
                    np.concatenate(
                        [
                            w_attn[:, g * 512 : (g + 1) * 512],
                            w_attn[:, C + g * 512 : C + (g + 1) * 512],
                        ],
                        axis=1,
                    ).astype(np.float16)
                ),
                "w_v": np.ascontiguousarray(
                    w_attn[:, 2 * C + g * 512 : 2 * C + (g + 1) * 512].astype(
                        np.float16
                    )
                ),
                "w_p": np.ascontiguousarray(w_p_full[g * 512 : (g + 1) * 512, :]),
                "lam": lam_tile,
            }
        )
    return in_maps


def assemble(results):
    y = np.empty((B, T, C), np.float32)
    for b in range(B):
        y[b] = results[b * G]["y"] + results[b * G + 1]["y"]
    return y


def kernel(**inputs) -> np.ndarray:
    nc = _get_nc()
    in_maps = make_in_maps(**inputs)
    res = run_bass_kernel_spmd(nc, in_maps, list(range(N_CORES)))
    return assemble(res.results)


# revision 23
# speedup vs baseline: 1.4232x; 1.0140x over previous
"""MultiHeadDiffAttn Trainium2 kernel (v2, 16-bit matmul path).

Sharding: 8 cores = 4-way data parallel over batch x 2-way tensor parallel
over heads (8 v-heads / 16 half-heads per core).  Each core computes its
batch's qkv projection restricted to its head group, differential attention
with per-half-head softmax, head RMS norm, and a partial output projection
(its 512 rows of w_proj).  Host sums the two partial projections per batch.

Measured device behavior this kernel is shaped around:
  - fp32 matmul streams at ~1/4 the 16-bit rate (and fp32 K=32 at 1/8), so
    all matmul operands are fp16 (PSUM accumulation stays fp32).
  - K=32 16-bit matmuls stream at 2 cycles/col; the two half-heads of a
    head sit at different 32-row strips of the PE array, so their S^T
    matmuls are emitted adjacently with explicit tile_position to overlap.
  - ACT exp costs ~0.83ns/col + ~360ns/op fixed, so exp runs once per
    (half-head, s-block) over the whole [128, up-to-1024] PSUM row-block.
  - AV runs transposed (U^T[dv,t] = v_aug^T @ expS^T) so it is
    stream-bound with a tiny reused LDWEIGHTS; a ones-column appended to v
    makes row 64 of U^T the softmax denominator.  The den row is
    reciprocal'd in SBUF before PE-transposing U^T back to [t, dv+1], so
    normalization + lambda-combine + RMS are native per-partition ops.
"""

import math
from contextlib import ExitStack

import numpy as np

import concourse.bass as bass
import concourse.tile as tile
from concourse import masks, mybir
from concourse.bass_utils import run_bass_kernel_spmd

# The deployed walrus rejects instructions carrying more than one sync wait
# ("Too many sync wait commands" in setupSyncWait).  Legalize at the BIR-JSON
# level: for every instruction with >1 wait, hoist the extra waits onto NoOp
# instructions inserted just before it on the same engine (engine streams are
# in-order, so semantics are identical).
_MAX_WAITS = 1


def _legalize_sync_waits(d):
    for f in d.get("functions", []):
        for bb in f.get("blocks", []):
            out = []
            for inst in bb["instructions"]:
                si = inst.get("sync_info")
                waits = (si or {}).get("on_wait") or []
                if len(waits) > _MAX_WAITS:
                    extra = waits[: len(waits) - _MAX_WAITS]
                    keep = waits[len(waits) - _MAX_WAITS :]
                    for j in range(0, len(extra), _MAX_WAITS):
                        nop = {
                            "engine": inst["engine"],
                            "ins": [],
                            "outs": [],
                            "name": f"{inst['name']}-lw{j}",
                            "opcode": "NoOp",
                            "sync_info": {
                                "on_wait": extra[j : j + _MAX_WAITS],
                                "on_update": [],
                            },
                        }
                        if "debug" in inst:
                            nop["debug"] = inst["debug"]
                        out.append(nop)
                    si["on_wait"] = keep
                out.append(inst)
            bb["instructions"] = out
    return d


_orig_to_json_bytes = bass.Bass.to_json_bytes


def _patched_to_json_bytes(self, *a, **kw):
    import json as _json

    raw = _orig_to_json_bytes(self, *a, **kw)
    return _json.dumps(_legalize_sync_waits(_json.loads(raw))).encode()


bass.Bass.to_json_bytes = _patched_to_json_bytes

F32 = mybir.dt.float32
F16 = mybir.dt.float16

B, T, C = 4, 1024, 1024
H_TOT = 16  # total v-heads
HD = 32  # half-head dim
DV = 64  # v-head dim
G = 2  # head groups (tensor parallel)
HPG = H_TOT // G  # 8 v-heads per core
COLS = 1024  # q cols + k cols per group
LAMBDA_INIT = 0.8 - 0.6 * math.exp(-0.3 * (1 - 1))  # 0.2
EPS = 1e-5
N_CORES = 8

NT = T // 128  # 8 t-tiles
NKC = C // 128  # 8 contraction chunks


def _emit(ctx: ExitStack, tc: tile.TileContext, xT, w_qk, w_v, w_p, lam, y):
    nc = tc.nc
    AluOp = mybir.AluOpType
    Act = mybir.ActivationFunctionType

    const = ctx.enter_context(tc.tile_pool(name="const", bufs=1))
    ident = const.tile([128, 128], F16)
    masks.make_identity(nc, ident[:])
    lam_sb = const.tile([128, 1], F32)
    nc.sync.dma_start(out=lam_sb, in_=lam[:])
    eps_sb = const.tile([128, 1], F32)
    nc.vector.memset(eps_sb, EPS)

    big = ctx.enter_context(tc.tile_pool(name="big", bufs=1))
    qkT_sb = big.tile([128, 8, T], F16)  # row-chunks of [COLS, T]
    v_sb = big.tile([128, NT, HPG, 128], F16)  # [s-chunk][head][dv | ones | 0-pad]
    outcat_sb = big.tile([128, NT, HPG * DV], F16)  # [t-chunk][512]
    outcatT_sb = big.tile([128, 4, T], F16)  # row-chunks of [512, T]
    wp_sb = big.tile([128, 4, C], F16)
    # per-half-head q, zero-padded to K=128: data lives at the same 32-row
    # strip as that half-head's k rows inside its qkT chunk, so the S^T
    # matmul can contract over the full 128 partitions at full stream rate
    # (the other half-heads' k rows meet zero q rows).
    qTp_sb = big.tile([128, 2 * HPG, T], F16)

    nc.gpsimd.memset(v_sb[:, :, :, DV + 1 :], 0.0)
    nc.gpsimd.memset(qTp_sb, 0.0)

    # ---------------- phase 1+2: qkv projections ----------------
    with (
        tc.tile_pool(name="xw", bufs=1) as xw,
        tc.tile_pool(name="mmps", bufs=4, space="PSUM") as mmps,
    ):
        xT_sb = xw.tile([128, NKC, T], F16)
        wqk_sb = xw.tile([128, NKC, COLS], F16)
        wv_sb = xw.tile([128, NKC, 512], F16)

        xT_r = xT[:].rearrange("(c p) t -> p c t", p=128)
        wqk_r = w_qk[:].rearrange("(c p) m -> p c m", p=128)

        def load_wqk(cc):
            nc.sync.dma_start(
                out=wqk_sb[:, :, cc * 128 : (cc + 1) * 128],
                in_=wqk_r[:, :, cc * 128 : (cc + 1) * 128],
            )

        load_wqk(0)
        for nh in range(2):  # t-halves so first matmuls start early
            nc.sync.dma_start(
                out=xT_sb[:, :, nh * 512 : (nh + 1) * 512],
                in_=xT_r[:, :, nh * 512 : (nh + 1) * 512],
            )
        for cc in range(1, 8):
            load_wqk(cc)
        nc.sync.dma_start(out=wv_sb, in_=w_v[:].rearrange("(c p) m -> p c m", p=128))
        nc.sync.dma_start(
            out=wp_sb, in_=w_p[:].rearrange("(c p) m -> p c m", p=128)
        )

        # qkT[cc-block, :] = w_qk[:, cc-block].T @ x^T
        for cc in range(8):
            for nh in range(2):
                ps = mmps.tile([128, 512], F32, tag="mmps", name=f"qk{cc}{nh}")
                for kc in range(NKC):
                    nc.tensor.matmul(
                        ps,
                        wqk_sb[:, kc, cc * 128 : (cc + 1) * 128],
                        xT_sb[:, kc, nh * 512 : (nh + 1) * 512],
                        start=(kc == 0),
                        stop=(kc == NKC - 1),
                    )
                nc.vector.tensor_copy(
                    out=qkT_sb[:, cc, nh * 512 : (nh + 1) * 512], in_=ps
                )
                if cc < 4:  # q chunk: scatter the 4 half-heads into qTp
                    for j in range(4):
                        hh = cc * 4 + j
                        nc.vector.tensor_copy(
                            out=qTp_sb[
                                j * 32 : (j + 1) * 32,
                                hh,
                                nh * 512 : (nh + 1) * 512,
                            ],
                            in_=qkT_sb[
                                j * 32 : (j + 1) * 32,
                                cc,
                                nh * 512 : (nh + 1) * 512,
                            ],
                        )

        # v[t-block, :] = x @ w_v ; scatter heads into v_sb, slot 64 = ones
        for tt in range(NT):
            ps = mmps.tile([128, 512], F32, tag="mmps", name=f"v{tt}")
            for kc in range(NKC):
                nc.tensor.matmul(
                    ps,
                    xT_sb[:, kc, tt * 128 : (tt + 1) * 128],
                    wv_sb[:, kc, :],
                    start=(kc == 0),
                    stop=(kc == NKC - 1),
                )
            nc.vector.tensor_copy(
                out=v_sb[:, tt, :, 0:DV],
                in_=ps[:].rearrange("p (h d) -> p h d", h=HPG),
            )
            nc.vector.memset(v_sb[:, tt, :, DV : DV + 1], 1.0)

    # ---------------- phase 3: differential attention ----------------
    with (
        tc.tile_pool(name="es", bufs=3) as es_pool,
        tc.tile_pool(name="us", bufs=2) as us_pool,
        tc.tile_pool(name="sps", bufs=1, space="PSUM") as s_pool,
        tc.tile_pool(name="ups", bufs=1, space="PSUM") as u_pool,
        tc.tile_pool(name="comb", bufs=3) as comb,
        tc.tile_pool(name="ohp", bufs=1, space="SBUF") as ohp,
    ):
        oh_all = {}
        ssq_all = ohp.tile([128, HPG * NT], F32, tag="ssqall", name="ssqall")

        def emit_av(h, s, e, es_s, chunks):
            # U[t-block, dv|den] += expS^T[s-block, t-block].T @ v_aug[s-block]
            for tj in range(s, NT):
                off = (tj % 4) * 128
                nc.tensor.matmul(
                    u_tiles[e][tj // 4][:, off : off + DV + 1],
                    es_s[:, tj * 128 : (tj + 1) * 128],
                    v_sb[:, s, h, 0 : DV + 1],
                    start=(s == 0 and tj % 4 == 0),
                    stop=(s == tj and tj % 4 == 3),
                )

        for h in range(HPG):
            qc = h // 2
            kc_ = 4 + h // 2
            pbase = [(2 * h % 4) * 32, (2 * h % 4) * 32 + 32]
            s_tiles = [
                s_pool.tile([128, T], F32, tag=f"s{e}", name=f"s{e}_{h}")
                for e in range(2)
            ]
            u_tiles = [
                [
                    u_pool.tile(
                        [128, 512], F32, tag=f"u{e}{b}", name=f"u{e}{b}_{h}"
                    )
                    for b in range(2)
                ]
                for e in range(2)
            ]
            prev = None  # both AVs delayed one s-iteration
            for s in range(NT):
                t0 = 128 * s
                chunks = [(t0, 512), (512, 1024)] if s < 4 else [(t0, 1024)]
                es_s = [
                    es_pool.tile([128, T], F16, tag=f"es{e}", name=f"es{e}_{h}_{s}")
                    for e in range(2)
                ]
                for c0, c1 in chunks:
                    for e in range(2):
                        nc.tensor.matmul(
                            s_tiles[e][:, c0:c1],
                            qkT_sb[:, kc_, t0 : t0 + 128],
                            qTp_sb[:, 2 * h + e, c0:c1],
                            start=True,
                            stop=True,
                        )
                if prev is not None:
                    ps_, pes_, pchunks_ = prev
                    emit_av(h, ps_, 0, pes_[0], pchunks_)
                    emit_av(h, ps_, 1, pes_[1], pchunks_)
                for e in range(2):
                    nc.scalar.activation(
                        out=es_s[e][:, t0:T],
                        in_=s_tiles[e][:, t0:T],
                        func=Act.Exp,
                        scale=1.0 / 32.0,
                    )
                    # causal mask inside the diagonal block: keep t >= s
                    nc.gpsimd.affine_select(
                        out=es_s[e][:, t0 : t0 + 128],
                        in_=es_s[e][:, t0 : t0 + 128],
                        pattern=[[1, 128]],
                        compare_op=AluOp.is_ge,
                        fill=0.0,
                        base=0,
                        channel_multiplier=-1,
                    )
                prev = (s, es_s, chunks)
            ps_, pes_, pchunks_ = prev
            emit_av(h, ps_, 0, pes_[0], pchunks_)
            emit_av(h, ps_, 1, pes_[1], pchunks_)

            # ---- normalize, lambda-combine, accumulate sum-of-squares ----
            for tj in range(NT):
                off = (tj % 4) * 128
                up = [u_tiles[e][tj // 4] for e in range(2)]
                rr_ = [
                    comb.tile([128, 1], F32, tag=f"r{e}", name=f"r{e}_{h}_{tj}")
                    for e in range(2)
                ]
                for e in range(2):
                    nc.vector.reciprocal(
                        out=rr_[e], in_=up[e][:, off + DV : off + DV + 1]
                    )
                t2 = comb.tile([128, DV], F32, tag="t2", name=f"t2_{h}_{tj}")
                nc.vector.tensor_scalar(
                    out=t2,
                    in0=up[1][:, off : off + DV],
                    scalar1=rr_[1][:],
                    scalar2=lam_sb[:],
                    op0=AluOp.mult,
                    op1=AluOp.mult,
                )
                oh = ohp.tile(
                    [128, DV], F32, tag=f"oh{h}_{tj}", name=f"oh_{h}_{tj}"
                )
                nc.vector.scalar_tensor_tensor(
                    out=oh,
                    in0=up[0][:, off : off + DV],
                    scalar=rr_[0][:],
                    in1=t2,
                    op0=AluOp.mult,
                    op1=AluOp.subtract,
                )
                sq = comb.tile([128, DV], F32, tag="sq", name=f"sq_{h}_{tj}")
                nc.vector.scalar_tensor_tensor(
                    out=sq,
                    in0=oh,
                    scalar=1.0,
                    in1=oh,
                    op0=AluOp.mult,
                    op1=AluOp.mult,
                    accum_out=ssq_all[:, h * NT + tj : h * NT + tj + 1],
                )
                oh_all[(h, tj)] = oh

        # ---- deferred RMS: one batched Sqrt for all heads ----
        rstd = comb.tile([128, HPG * NT], F32, tag="rstd", name="rstd")
        nc.scalar.activation(
            out=rstd, in_=ssq_all, func=Act.Sqrt, bias=eps_sb[:], scale=1.0 / DV
        )
        nc.vector.reciprocal(out=rstd, in_=rstd)
        for h in range(HPG):
            for tj in range(NT):
                nc.vector.tensor_scalar_mul(
                    out=outcat_sb[:, tj, h * DV : (h + 1) * DV],
                    in0=oh_all[(h, tj)],
                    scalar1=rstd[:, h * NT + tj : h * NT + tj + 1],
                )

    # ---------------- phase 4+5: transpose + output projection ----------------
    with (
        tc.tile_pool(name="tps", bufs=2, space="PSUM") as tps,
        tc.tile_pool(name="pps", bufs=4, space="PSUM") as pps,
        tc.tile_pool(name="yout", bufs=2) as yout,
    ):
        for rr in range(4):
            for tt in range(NT):
                ps = tps.tile([128, 128], F16, tag="tp", name=f"ot{rr}{tt}")
                nc.tensor.transpose(
                    ps, outcat_sb[:, tt, rr * 128 : (rr + 1) * 128], ident
                )
                nc.vector.tensor_copy(
                    out=outcatT_sb[:, rr, tt * 128 : (tt + 1) * 128], in_=ps
                )
        for tt in range(NT):
            yt = yout.tile([128, C], F32, tag="yt", name=f"y{tt}")
            for nh in range(2):
                ps = pps.tile([128, 512], F32, tag="pp", name=f"pp{tt}{nh}")
                for rr in range(4):
                    nc.tensor.matmul(
                        ps,
                        outcatT_sb[:, rr, tt * 128 : (tt + 1) * 128],
                        wp_sb[:, rr, nh * 512 : (nh + 1) * 512],
                        start=(rr == 0),
                        stop=(rr == 3),
                    )
                nc.vector.tensor_copy(out=yt[:, nh * 512 : (nh + 1) * 512], in_=ps)
            nc.sync.dma_start(out=y[tt * 128 : (tt + 1) * 128, :], in_=yt)


def build_nc():
    nc = bass.Bass()
    xT = nc.declare_dram_parameter("xT", [C, T], F16, isOutput=False)
    w_qk = nc.declare_dram_parameter("w_qk", [C, COLS], F16, isOutput=False)
    w_v = nc.declare_dram_parameter("w_v", [C, 512], F16, isOutput=False)
    w_p = nc.declare_dram_parameter("w_p", [512, C], F16, isOutput=False)
    lam = nc.declare_dram_parameter("lam", [128, 1], F32, isOutput=False)
    y = nc.declare_dram_parameter("y", [T, C], F32, isOutput=True)
    with tile.TileContext(nc) as tc:
        with ExitStack() as ctx:
            _emit(ctx, tc, xT, w_qk, w_v, w_p, lam, y)
    return nc


_NC = None


def _get_nc():
    global _NC
    if _NC is None:
        _NC = build_nc()
    return _NC


def make_in_maps(x, w_attn, w_proj, lambda_q1, lambda_q2, lambda_k1, lambda_k2, gamma):
    x = np.asarray(x, np.float32)
    w_attn = np.asarray(w_attn, np.float32)
    w_proj = np.asarray(w_proj, np.float32)
    lam1 = np.exp(np.sum(np.float32(lambda_q1) * np.float32(lambda_k1), dtype=np.float32))
    lam2 = np.exp(np.sum(np.float32(lambda_q2) * np.float32(lambda_k2), dtype=np.float32))
    lam_full = np.float32(lam1 - lam2 + LAMBDA_INIT)
    lam_tile = np.full((128, 1), lam_full, np.float32)
    # fold gamma * (1 - lambda_init) into w_proj rows
    scale = np.tile(np.asarray(gamma, np.float32), H_TOT) * np.float32(1.0 - LAMBDA_INIT)
    w_p_full = (w_proj * scale[:, None]).astype(np.float16)

    in_maps = []
    for core in range(N_CORES):
        b, g = core // G, core % G
        in_maps.append(
            {
                "xT": np.ascontiguousarray(x[b].T.astype(np.float16)),
                "w_qk": np.ascontiguousarray(
                    np.concatenate(
                        [
                            w_attn[:, g * 512 : (g + 1) * 512],
                            w_attn[:, C + g * 512 : C + (g + 1) * 512],
                        ],
                        axis=1,
                    ).astype(np.float16)
                ),
                "w_v": np.ascontiguousarray(
                    w_attn[:, 2 * C + g * 512 : 2 * C + (g + 1) * 512].astype(
                        np.float16
                    )
                ),
                "w_p": np.ascontiguousarray(w_p_full[g * 512 : (g + 1) * 512, :]),
                "lam": lam_tile,
            }
        )
    return in_maps


def assemble(results):
    y = np.empty((B, T, C), np.float32)
    for b in range(B):
        y[b] = results[b * G]["y"] + results[b * G + 1]["y"]
    return y


def kernel(**inputs) -> np.ndarray:
    nc = _get_nc()
    in_maps = make_in_maps(**inputs)
    res = run_bass_kernel_spmd(nc, in_maps, list(range(N_CORES)))
    return assemble(res.results)


# revision 25
# speedup vs baseline: 1.4255x; 1.0016x over previous
"""MultiHeadDiffAttn Trainium2 kernel (v2, 16-bit matmul path).

Sharding: 8 cores = 4-way data parallel over batch x 2-way tensor parallel
over heads (8 v-heads / 16 half-heads per core).  Each core computes its
batch's qkv projection restricted to its head group, differential attention
with per-half-head softmax, head RMS norm, and a partial output projection
(its 512 rows of w_proj).  Host sums the two partial projections per batch.

Measured device behavior this kernel is shaped around:
  - fp32 matmul streams at ~1/4 the 16-bit rate (and fp32 K=32 at 1/8), so
    all matmul operands are fp16 (PSUM accumulation stays fp32).
  - K=32 16-bit matmuls stream at 2 cycles/col; the two half-heads of a
    head sit at different 32-row strips of the PE array, so their S^T
    matmuls are emitted adjacently with explicit tile_position to overlap.
  - ACT exp costs ~0.83ns/col + ~360ns/op fixed, so exp runs once per
    (half-head, s-block) over the whole [128, up-to-1024] PSUM row-block.
  - AV runs transposed (U^T[dv,t] = v_aug^T @ expS^T) so it is
    stream-bound with a tiny reused LDWEIGHTS; a ones-column appended to v
    makes row 64 of U^T the softmax denominator.  The den row is
    reciprocal'd in SBUF before PE-transposing U^T back to [t, dv+1], so
    normalization + lambda-combine + RMS are native per-partition ops.
"""

import math
from contextlib import ExitStack

import numpy as np

import concourse.bass as bass
import concourse.tile as tile
from concourse import masks, mybir
from concourse.bass_utils import run_bass_kernel_spmd

# The deployed walrus rejects instructions carrying more than one sync wait
# ("Too many sync wait commands" in setupSyncWait).  Legalize at the BIR-JSON
# level: for every instruction with >1 wait, hoist the extra waits onto NoOp
# instructions inserted just before it on the same engine (engine streams are
# in-order, so semantics are identical).
_MAX_WAITS = 1


def _legalize_sync_waits(d):
    for f in d.get("functions", []):
        for bb in f.get("blocks", []):
            out = []
            for inst in bb["instructions"]:
                si = inst.get("sync_info")
                waits = (si or {}).get("on_wait") or []
                if len(waits) > _MAX_WAITS:
                    extra = waits[: len(waits) - _MAX_WAITS]
                    keep = waits[len(waits) - _MAX_WAITS :]
                    for j in range(0, len(extra), _MAX_WAITS):
                        nop = {
                            "engine": inst["engine"],
                            "ins": [],
                            "outs": [],
                            "name": f"{inst['name']}-lw{j}",
                            "opcode": "NoOp",
                            "sync_info": {
                                "on_wait": extra[j : j + _MAX_WAITS],
                                "on_update": [],
                            },
                        }
                        if "debug" in inst:
                            nop["debug"] = inst["debug"]
                        out.append(nop)
                    si["on_wait"] = keep
                out.append(inst)
            bb["instructions"] = out
    return d


_orig_to_json_bytes = bass.Bass.to_json_bytes


def _patched_to_json_bytes(self, *a, **kw):
    import json as _json

    raw = _orig_to_json_bytes(self, *a, **kw)
    return _json.dumps(_legalize_sync_waits(_json.loads(raw))).encode()


bass.Bass.to_json_bytes = _patched_to_json_bytes

F32 = mybir.dt.float32
F16 = mybir.dt.float16

B, T, C = 4, 1024, 1024
H_TOT = 16  # total v-heads
HD = 32  # half-head dim
DV = 64  # v-head dim
G = 2  # head groups (tensor parallel)
HPG = H_TOT // G  # 8 v-heads per core
COLS = 1024  # q cols + k cols per group
LAMBDA_INIT = 0.8 - 0.6 * math.exp(-0.3 * (1 - 1))  # 0.2
EPS = 1e-5
N_CORES = 8

NT = T // 128  # 8 t-tiles
NKC = C // 128  # 8 contraction chunks


def _emit(ctx: ExitStack, tc: tile.TileContext, xT, w_qk, w_v, w_p, lam, y):
    nc = tc.nc
    AluOp = mybir.AluOpType
    Act = mybir.ActivationFunctionType

    const = ctx.enter_context(tc.tile_pool(name="const", bufs=1))
    ident = const.tile([128, 128], F16)
    masks.make_identity(nc, ident[:])
    lam_sb = const.tile([128, 1], F32)
    nc.sync.dma_start(out=lam_sb, in_=lam[:])
    eps_sb = const.tile([128, 1], F32)
    nc.vector.memset(eps_sb, EPS)

    big = ctx.enter_context(tc.tile_pool(name="big", bufs=1))
    qkT_sb = big.tile([128, 8, T], F16)  # row-chunks of [COLS, T]
    v_sb = big.tile([128, NT, HPG, 128], F16)  # [s-chunk][head][dv | ones | 0-pad]
    outcat_sb = big.tile([128, NT, HPG * DV], F16)  # [t-chunk][512]
    outcatT_sb = big.tile([128, 4, T], F16)  # row-chunks of [512, T]
    wp_sb = big.tile([128, 4, C], F16)
    # per-half-head q, zero-padded to K=128: data lives at the same 32-row
    # strip as that half-head's k rows inside its qkT chunk, so the S^T
    # matmul can contract over the full 128 partitions at full stream rate
    # (the other half-heads' k rows meet zero q rows).
    qTp_sb = big.tile([128, 2 * HPG, T], F16)

    nc.gpsimd.memset(v_sb[:, :, :, DV + 1 :], 0.0)
    nc.gpsimd.memset(qTp_sb, 0.0)

    # ---------------- phase 1+2: qkv projections ----------------
    with (
        tc.tile_pool(name="xw", bufs=1) as xw,
        tc.tile_pool(name="mmps", bufs=4, space="PSUM") as mmps,
    ):
        xT_sb = xw.tile([128, NKC, T], F16)
        wqk_sb = xw.tile([128, NKC, COLS], F16)
        wv_sb = xw.tile([128, NKC, 512], F16)

        xT_r = xT[:].rearrange("(c p) t -> p c t", p=128)
        wqk_r = w_qk[:].rearrange("(c p) m -> p c m", p=128)

        def load_wqk(cc):
            nc.sync.dma_start(
                out=wqk_sb[:, :, cc * 128 : (cc + 1) * 128],
                in_=wqk_r[:, :, cc * 128 : (cc + 1) * 128],
            )

        load_wqk(0)
        for nh in range(2):  # t-halves so first matmuls start early
            nc.sync.dma_start(
                out=xT_sb[:, :, nh * 512 : (nh + 1) * 512],
                in_=xT_r[:, :, nh * 512 : (nh + 1) * 512],
            )
        for cc in range(1, 8):
            load_wqk(cc)
        nc.sync.dma_start(out=wv_sb, in_=w_v[:].rearrange("(c p) m -> p c m", p=128))
        nc.sync.dma_start(
            out=wp_sb, in_=w_p[:].rearrange("(c p) m -> p c m", p=128)
        )

        # qkT[cc-block, :] = w_qk[:, cc-block].T @ x^T
        for cc in range(8):
            for nh in range(2):
                ps = mmps.tile([128, 512], F32, tag="mmps", name=f"qk{cc}{nh}")
                for kc in range(NKC):
                    nc.tensor.matmul(
                        ps,
                        wqk_sb[:, kc, cc * 128 : (cc + 1) * 128],
                        xT_sb[:, kc, nh * 512 : (nh + 1) * 512],
                        start=(kc == 0),
                        stop=(kc == NKC - 1),
                    )
                nc.vector.tensor_copy(
                    out=qkT_sb[:, cc, nh * 512 : (nh + 1) * 512], in_=ps
                )
                if cc < 4:  # q chunk: scatter the 4 half-heads into qTp
                    for j in range(4):
                        hh = cc * 4 + j
                        nc.vector.tensor_copy(
                            out=qTp_sb[
                                j * 32 : (j + 1) * 32,
                                hh,
                                nh * 512 : (nh + 1) * 512,
                            ],
                            in_=qkT_sb[
                                j * 32 : (j + 1) * 32,
                                cc,
                                nh * 512 : (nh + 1) * 512,
                            ],
                        )

        # v[t-block, :] = x @ w_v ; scatter heads into v_sb, slot 64 = ones
        for tt in range(NT):
            ps = mmps.tile([128, 512], F32, tag="mmps", name=f"v{tt}")
            for kc in range(NKC):
                nc.tensor.matmul(
                    ps,
                    xT_sb[:, kc, tt * 128 : (tt + 1) * 128],
                    wv_sb[:, kc, :],
                    start=(kc == 0),
                    stop=(kc == NKC - 1),
                )
            nc.vector.tensor_copy(
                out=v_sb[:, tt, :, 0:DV],
                in_=ps[:].rearrange("p (h d) -> p h d", h=HPG),
            )
            nc.vector.memset(v_sb[:, tt, :, DV : DV + 1], 1.0)

    # ---------------- phase 3: differential attention ----------------
    with (
        tc.tile_pool(name="es", bufs=3) as es_pool,
        tc.tile_pool(name="us", bufs=2) as us_pool,
        tc.tile_pool(name="sps", bufs=1, space="PSUM") as s_pool,
        tc.tile_pool(name="ups", bufs=1, space="PSUM") as u_pool,
        tc.tile_pool(name="comb", bufs=3) as comb,
        tc.tile_pool(name="ohp", bufs=1, space="SBUF") as ohp,
    ):
        oh_all = {}
        ssq_all = ohp.tile([128, HPG * NT], F32, tag="ssqall", name="ssqall")

        def emit_av(h, s, e, es_s, chunks):
            # U[t-block, dv|den] += expS^T[s-block, t-block].T @ v_aug[s-block]
            for tj in range(s, NT):
                off = (tj % 4) * 128
                nc.tensor.matmul(
                    u_tiles[e][tj // 4][:, off : off + DV + 1],
                    es_s[:, tj * 128 : (tj + 1) * 128],
                    v_sb[:, s, h, 0 : DV + 1],
                    start=(s == 0 and tj % 4 == 0),
                    stop=(s == tj and tj % 4 == 3),
                )

        # ---- RMS in two batches: heads 0-3 overlap attention of heads 4-7
        def emit_rms(h_lo, h_hi):
            w = (h_hi - h_lo) * NT
            rstd = comb.tile(
                [128, w], F32, tag=f"rstd{h_lo}", name=f"rstd{h_lo}"
            )
            nc.scalar.activation(
                out=rstd,
                in_=ssq_all[:, h_lo * NT : h_hi * NT],
                func=Act.Sqrt,
                bias=eps_sb[:],
                scale=1.0 / DV,
            )
            nc.vector.reciprocal(out=rstd, in_=rstd)
            for h in range(h_lo, h_hi):
                for tj in range(NT):
                    nc.vector.tensor_scalar_mul(
                        out=outcat_sb[:, tj, h * DV : (h + 1) * DV],
                        in0=oh_all[(h, tj)],
                        scalar1=rstd[:, (h - h_lo) * NT + tj : (h - h_lo) * NT + tj + 1],
                    )


        for h in range(HPG):
            qc = h // 2
            kc_ = 4 + h // 2
            pbase = [(2 * h % 4) * 32, (2 * h % 4) * 32 + 32]
            s_tiles = [
                s_pool.tile([128, T], F32, tag=f"s{e}", name=f"s{e}_{h}")
                for e in range(2)
            ]
            u_tiles = [
                [
                    u_pool.tile(
                        [128, 512], F32, tag=f"u{e}{b}", name=f"u{e}{b}_{h}"
                    )
                    for b in range(2)
                ]
                for e in range(2)
            ]
            prev = None  # both AVs delayed one s-iteration
            for s in range(NT):
                t0 = 128 * s
                chunks = [(t0, 512), (512, 1024)] if s < 4 else [(t0, 1024)]
                es_s = [
                    es_pool.tile([128, T], F16, tag=f"es{e}", name=f"es{e}_{h}_{s}")
                    for e in range(2)
                ]
                for c0, c1 in chunks:
                    for e in range(2):
                        nc.tensor.matmul(
                            s_tiles[e][:, c0:c1],
                            qkT_sb[:, kc_, t0 : t0 + 128],
                            qTp_sb[:, 2 * h + e, c0:c1],
                            start=True,
                            stop=True,
                        )
                if prev is not None:
                    ps_, pes_, pchunks_ = prev
                    emit_av(h, ps_, 0, pes_[0], pchunks_)
                    emit_av(h, ps_, 1, pes_[1], pchunks_)
                for e in range(2):
                    nc.scalar.activation(
                        out=es_s[e][:, t0:T],
                        in_=s_tiles[e][:, t0:T],
                        func=Act.Exp,
                        scale=1.0 / 32.0,
                    )
                    # causal mask inside the diagonal block: keep t >= s
                    nc.gpsimd.affine_select(
                        out=es_s[e][:, t0 : t0 + 128],
                        in_=es_s[e][:, t0 : t0 + 128],
                        pattern=[[1, 128]],
                        compare_op=AluOp.is_ge,
                        fill=0.0,
                        base=0,
                        channel_multiplier=-1,
                    )
                prev = (s, es_s, chunks)
            ps_, pes_, pchunks_ = prev
            emit_av(h, ps_, 0, pes_[0], pchunks_)
            emit_av(h, ps_, 1, pes_[1], pchunks_)

            # ---- normalize, lambda-combine, accumulate sum-of-squares ----
            for tj in range(NT):
                off = (tj % 4) * 128
                up = [u_tiles[e][tj // 4] for e in range(2)]
                rr_ = [
                    comb.tile([128, 1], F32, tag=f"r{e}", name=f"r{e}_{h}_{tj}")
                    for e in range(2)
                ]
                for e in range(2):
                    nc.vector.reciprocal(
                        out=rr_[e], in_=up[e][:, off + DV : off + DV + 1]
                    )
                t2 = comb.tile([128, DV], F32, tag="t2", name=f"t2_{h}_{tj}")
                nc.vector.tensor_scalar(
                    out=t2,
                    in0=up[1][:, off : off + DV],
                    scalar1=rr_[1][:],
                    scalar2=lam_sb[:],
                    op0=AluOp.mult,
                    op1=AluOp.mult,
                )
                oh = ohp.tile(
                    [128, DV], F32, tag=f"oh{h}_{tj}", name=f"oh_{h}_{tj}"
                )
                nc.vector.scalar_tensor_tensor(
                    out=oh,
                    in0=up[0][:, off : off + DV],
                    scalar=rr_[0][:],
                    in1=t2,
                    op0=AluOp.mult,
                    op1=AluOp.subtract,
                )
                sq = comb.tile([128, DV], F32, tag="sq", name=f"sq_{h}_{tj}")
                nc.vector.scalar_tensor_tensor(
                    out=sq,
                    in0=oh,
                    scalar=1.0,
                    in1=oh,
                    op0=AluOp.mult,
                    op1=AluOp.mult,
                    accum_out=ssq_all[:, h * NT + tj : h * NT + tj + 1],
                )
                oh_all[(h, tj)] = oh
            if h == 3:
                emit_rms(0, 4)

        emit_rms(4, HPG)

    # ---------------- phase 4+5: transpose + output projection ----------------
    with (
        tc.tile_pool(name="tps", bufs=2, space="PSUM") as tps,
        tc.tile_pool(name="pps", bufs=4, space="PSUM") as pps,
        tc.tile_pool(name="yout", bufs=2) as yout,
    ):
        for tt in range(NT):
            nc.sync.dma_start_transpose(
                out=outcatT_sb[:, :, tt * 128 : (tt + 1) * 128],
                in_=outcat_sb[:, tt, :],
            )
        for tt in range(NT):
            yt = yout.tile([128, C], F32, tag="yt", name=f"y{tt}")
            for nh in range(2):
                ps = pps.tile([128, 512], F32, tag="pp", name=f"pp{tt}{nh}")
                for rr in range(4):
                    nc.tensor.matmul(
                        ps,
                        outcatT_sb[:, rr, tt * 128 : (tt + 1) * 128],
                        wp_sb[:, rr, nh * 512 : (nh + 1) * 512],
                        start=(rr == 0),
                        stop=(rr == 3),
                    )
                nc.vector.tensor_copy(out=yt[:, nh * 512 : (nh + 1) * 512], in_=ps)
            nc.sync.dma_start(out=y[tt * 128 : (tt + 1) * 128, :], in_=yt)


def build_nc():
    nc = bass.Bass()
    xT = nc.declare_dram_parameter("xT", [C, T], F16, isOutput=False)
    w_qk = nc.declare_dram_parameter("w_qk", [C, COLS], F16, isOutput=False)
    w_v = nc.declare_dram_parameter("w_v", [C, 512], F16, isOutput=False)
    w_p = nc.declare_dram_parameter("w_p", [512, C], F16, isOutput=False)
    lam = nc.declare_dram_parameter("lam", [128, 1], F32, isOutput=False)
    y = nc.declare_dram_parameter("y", [T, C], F32, isOutput=True)
    with tile.TileContext(nc) as tc:
        with ExitStack() as ctx:
            _emit(ctx, tc, xT, w_qk, w_v, w_p, lam, y)
    return nc


_NC = None


def _get_nc():
    global _NC
    if _NC is None:
        _NC = build_nc()
    return _NC


def make_in_maps(x, w_attn, w_proj, lambda_q1, lambda_q2, lambda_k1, lambda_k2, gamma):
    x = np.asarray(x, np.float32)
    w_attn = np.asarray(w_attn, np.float32)
    w_proj = np.asarray(w_proj, np.float32)
    lam1 = np.exp(np.sum(np.float32(lambda_q1) * np.float32(lambda_k1), dtype=np.float32))
    lam2 = np.exp(np.sum(np.float32(lambda_q2) * np.float32(lambda_k2), dtype=np.float32))
    lam_full = np.float32(lam1 - lam2 + LAMBDA_INIT)
    lam_tile = np.full((128, 1), lam_full, np.float32)
    # fold gamma * (1 - lambda_init) into w_proj rows
    scale = np.tile(np.asarray(gamma, np.float32), H_TOT) * np.float32(1.0 - LAMBDA_INIT)
    w_p_full = (w_proj * scale[:, None]).astype(np.float16)

    in_maps = []
    for core in range(N_CORES):
        b, g = core // G, core % G
        in_maps.append(
            {
                "xT": np.ascontiguousarray(x[b].T.astype(np.float16)),
                "w_qk": np.ascontiguousarray(
                    np.concatenate(
                        [
                            w_attn[:, g * 512 : (g + 1) * 512],
                            w_attn[:, C + g * 512 : C + (g + 1) * 512],
                        ],
                        axis=1,
                    ).astype(np.float16)
                ),
                "w_v": np.ascontiguousarray(
                    w_attn[:, 2 * C + g * 512 : 2 * C + (g + 1) * 512].astype(
                        np.float16
                    )
                ),
                "w_p": np.ascontiguousarray(w_p_full[g * 512 : (g + 1) * 512, :]),
                "lam": lam_tile,
            }
        )
    return in_maps


def assemble(results):
    y = np.empty((B, T, C), np.float32)
    for b in range(B):
        y[b] = results[b * G]["y"] + results[b * G + 1]["y"]
    return y


def kernel(**inputs) -> np.ndarray:
    nc = _get_nc()
    in_maps = make_in_maps(**inputs)
    res = run_bass_kernel_spmd(nc, in_maps, list(range(N_CORES)))
    return assemble(res.results)


# revision 27
# speedup vs baseline: 1.4542x; 1.0201x over previous
"""MultiHeadDiffAttn Trainium2 kernel (v2, 16-bit matmul path).

Sharding: 8 cores = 4-way data parallel over batch x 2-way tensor parallel
over heads (8 v-heads / 16 half-heads per core).  Each core computes its
batch's qkv projection restricted to its head group, differential attention
with per-half-head softmax, head RMS norm, and a partial output projection
(its 512 rows of w_proj).  Host sums the two partial projections per batch.

Measured device behavior this kernel is shaped around:
  - fp32 matmul streams at ~1/4 the 16-bit rate (and fp32 K=32 at 1/8), so
    all matmul operands are fp16 (PSUM accumulation stays fp32).
  - K=32 16-bit matmuls stream at 2 cycles/col; the two half-heads of a
    head sit at different 32-row strips of the PE array, so their S^T
    matmuls are emitted adjacently with explicit tile_position to overlap.
  - ACT exp costs ~0.83ns/col + ~360ns/op fixed, so exp runs once per
    (half-head, s-block) over the whole [128, up-to-1024] PSUM row-block.
  - AV runs transposed (U^T[dv,t] = v_aug^T @ expS^T) so it is
    stream-bound with a tiny reused LDWEIGHTS; a ones-column appended to v
    makes row 64 of U^T the softmax denominator.  The den row is
    reciprocal'd in SBUF before PE-transposing U^T back to [t, dv+1], so
    normalization + lambda-combine + RMS are native per-partition ops.
"""

import math
from contextlib import ExitStack

import numpy as np

import concourse.bass as bass
import concourse.tile as tile
from concourse import masks, mybir
from concourse.bass_utils import run_bass_kernel_spmd

# The deployed walrus rejects instructions carrying more than one sync wait
# ("Too many sync wait commands" in setupSyncWait).  Legalize at the BIR-JSON
# level: for every instruction with >1 wait, hoist the extra waits onto NoOp
# instructions inserted just before it on the same engine (engine streams are
# in-order, so semantics are identical).
_MAX_WAITS = 1


def _legalize_sync_waits(d):
    for f in d.get("functions", []):
        for bb in f.get("blocks", []):
            out = []
            for inst in bb["instructions"]:
                si = inst.get("sync_info")
                waits = (si or {}).get("on_wait") or []
                if len(waits) > _MAX_WAITS:
                    extra = waits[: len(waits) - _MAX_WAITS]
                    keep = waits[len(waits) - _MAX_WAITS :]
                    for j in range(0, len(extra), _MAX_WAITS):
                        nop = {
                            "engine": inst["engine"],
                            "ins": [],
                            "outs": [],
                            "name": f"{inst['name']}-lw{j}",
                            "opcode": "NoOp",
                            "sync_info": {
                                "on_wait": extra[j : j + _MAX_WAITS],
                                "on_update": [],
                            },
                        }
                        if "debug" in inst:
                            nop["debug"] = inst["debug"]
                        out.append(nop)
                    si["on_wait"] = keep
                out.append(inst)
            bb["instructions"] = out
    return d


_orig_to_json_bytes = bass.Bass.to_json_bytes


def _patched_to_json_bytes(self, *a, **kw):
    import json as _json

    raw = _orig_to_json_bytes(self, *a, **kw)
    return _json.dumps(_legalize_sync_waits(_json.loads(raw))).encode()


bass.Bass.to_json_bytes = _patched_to_json_bytes

F32 = mybir.dt.float32
F16 = mybir.dt.float16

B, T, C = 4, 1024, 1024
H_TOT = 16  # total v-heads
HD = 32  # half-head dim
DV = 64  # v-head dim
G = 2  # head groups (tensor parallel)
HPG = H_TOT // G  # 8 v-heads per core
COLS = 1024  # q cols + k cols per group
LAMBDA_INIT = 0.8 - 0.6 * math.exp(-0.3 * (1 - 1))  # 0.2
EPS = 1e-5
N_CORES = 8

NT = T // 128  # 8 t-tiles
NKC = C // 128  # 8 contraction chunks


def _emit(ctx: ExitStack, tc: tile.TileContext, xT, w_qk, w_v, w_p, lam, y):
    nc = tc.nc
    AluOp = mybir.AluOpType
    Act = mybir.ActivationFunctionType

    const = ctx.enter_context(tc.tile_pool(name="const", bufs=1))
    ident = const.tile([128, 128], F16)
    masks.make_identity(nc, ident[:])
    lam_sb = const.tile([128, 1], F32)
    nc.sync.dma_start(out=lam_sb, in_=lam[:])
    eps_sb = const.tile([128, 1], F32)
    nc.vector.memset(eps_sb, EPS)

    big = ctx.enter_context(tc.tile_pool(name="big", bufs=1))
    qkT_sb = big.tile([128, 8, T], F16)  # row-chunks of [COLS, T]
    v_sb = big.tile([128, NT, HPG, 128], F16)  # [s-chunk][head][dv | ones | 0-pad]
    outcat_sb = big.tile([128, NT, HPG * DV], F16)  # [t-chunk][512]
    outcatT_sb = big.tile([128, 4, T], F16)  # row-chunks of [512, T]
    wp_sb = big.tile([128, 4, C], F16)
    # per-half-head q, zero-padded to K=128: data lives at the same 32-row
    # strip as that half-head's k rows inside its qkT chunk, so the S^T
    # matmul can contract over the full 128 partitions at full stream rate
    # (the other half-heads' k rows meet zero q rows).
    qTp_sb = big.tile([128, 2 * HPG, T], F16)

    nc.gpsimd.memset(v_sb[:, :, :, DV + 1 :], 0.0)
    es_pool = ctx.enter_context(tc.tile_pool(name="es", bufs=3))
    nc.gpsimd.memset(qTp_sb, 0.0)

    # ---------------- phase 1+2: qkv projections ----------------
    with (
        tc.tile_pool(name="xw", bufs=1) as xw,
        tc.tile_pool(name="mmps", bufs=4, space="PSUM") as mmps,
    ):
        xT_sb = xw.tile([128, NKC, T], F16)
        wqk_sb = xw.tile([128, NKC, COLS], F16)
        wv_sb = xw.tile([128, NKC, 512], F16)

        xT_r = xT[:].rearrange("(c p) t -> p c t", p=128)
        wqk_r = w_qk[:].rearrange("(c p) m -> p c m", p=128)

        def load_wqk(cc):
            nc.sync.dma_start(
                out=wqk_sb[:, :, cc * 128 : (cc + 1) * 128],
                in_=wqk_r[:, :, cc * 128 : (cc + 1) * 128],
            )

        load_wqk(0)
        for nh in range(2):  # t-halves so first matmuls start early
            nc.sync.dma_start(
                out=xT_sb[:, :, nh * 512 : (nh + 1) * 512],
                in_=xT_r[:, :, nh * 512 : (nh + 1) * 512],
            )
        for cc in range(1, 8):
            load_wqk(cc)
        nc.sync.dma_start(out=wv_sb, in_=w_v[:].rearrange("(c p) m -> p c m", p=128))
        nc.sync.dma_start(
            out=wp_sb, in_=w_p[:].rearrange("(c p) m -> p c m", p=128)
        )

        # qkT[cc-block, :] = w_qk[:, cc-block].T @ x^T
        for cc in range(8):
            for nh in range(2):
                ps = mmps.tile([128, 1024], F32, tag="mmps", name=f"qk{cc}{nh}")[:, 0:512]
                for kc in range(NKC):
                    nc.tensor.matmul(
                        ps,
                        wqk_sb[:, kc, cc * 128 : (cc + 1) * 128],
                        xT_sb[:, kc, nh * 512 : (nh + 1) * 512],
                        start=(kc == 0),
                        stop=(kc == NKC - 1),
                    )
                nc.vector.tensor_copy(
                    out=qkT_sb[:, cc, nh * 512 : (nh + 1) * 512], in_=ps
                )
                if cc < 4:  # q chunk: scatter the 4 half-heads into qTp
                    for j in range(4):
                        hh = cc * 4 + j
                        nc.vector.tensor_copy(
                            out=qTp_sb[
                                j * 32 : (j + 1) * 32,
                                hh,
                                nh * 512 : (nh + 1) * 512,
                            ],
                            in_=qkT_sb[
                                j * 32 : (j + 1) * 32,
                                cc,
                                nh * 512 : (nh + 1) * 512,
                            ],
                        )

        # prebake head 0, s=0..1: S/exp/mask run during the v projection
        prebaked = []
        for s in range(2):
            t0 = 128 * s
            chunks = [(t0, 512), (512, 1024)]
            es_s = [
                es_pool.tile([128, T], F16, tag=f"es{e}", name=f"esp{e}_{s}")
                for e in range(2)
            ]
            for e in range(2):
                sps = mmps.tile([128, 1024], F32, tag="mmps", name=f"sp{e}_{s}")
                for c0, c1 in chunks:
                    nc.tensor.matmul(
                        sps[:, c0:c1],
                        qkT_sb[:, 4, t0 : t0 + 128],
                        qTp_sb[:, e, c0:c1],
                        start=True,
                        stop=True,
                    )
                nc.scalar.activation(
                    out=es_s[e][:, t0:T],
                    in_=sps[:, t0:T],
                    func=Act.Exp,
                    scale=1.0 / 32.0,
                )
                nc.gpsimd.affine_select(
                    out=es_s[e][:, t0 : t0 + 128],
                    in_=es_s[e][:, t0 : t0 + 128],
                    pattern=[[1, 128]],
                    compare_op=AluOp.is_ge,
                    fill=0.0,
                    base=0,
                    channel_multiplier=-1,
                )
            prebaked.append((s, es_s, chunks))

        # v[t-block, :] = x @ w_v ; scatter heads into v_sb, slot 64 = ones
        for tt in range(NT):
            ps = mmps.tile([128, 1024], F32, tag="mmps", name=f"v{tt}")[:, 0:512]
            for kc in range(NKC):
                nc.tensor.matmul(
                    ps,
                    xT_sb[:, kc, tt * 128 : (tt + 1) * 128],
                    wv_sb[:, kc, :],
                    start=(kc == 0),
                    stop=(kc == NKC - 1),
                )
            nc.vector.tensor_copy(
                out=v_sb[:, tt, :, 0:DV],
                in_=ps[:].rearrange("p (h d) -> p h d", h=HPG),
            )
            nc.vector.memset(v_sb[:, tt, :, DV : DV + 1], 1.0)

    # ---------------- phase 3: differential attention ----------------
    with (
        tc.tile_pool(name="us", bufs=2) as us_pool,
        tc.tile_pool(name="sps", bufs=1, space="PSUM") as s_pool,
        tc.tile_pool(name="ups", bufs=1, space="PSUM") as u_pool,
        tc.tile_pool(name="comb", bufs=3) as comb,
        tc.tile_pool(name="ohp", bufs=1, space="SBUF") as ohp,
    ):
        oh_all = {}
        ssq_all = ohp.tile([128, HPG * NT], F32, tag="ssqall", name="ssqall")

        def emit_av(h, s, e, es_s, chunks):
            # U[t-block, dv|den] += expS^T[s-block, t-block].T @ v_aug[s-block]
            for tj in range(s, NT):
                off = (tj % 4) * 128
                nc.tensor.matmul(
                    u_tiles[e][tj // 4][:, off : off + DV + 1],
                    es_s[:, tj * 128 : (tj + 1) * 128],
                    v_sb[:, s, h, 0 : DV + 1],
                    start=(s == 0 and tj % 4 == 0),
                    stop=(s == tj and tj % 4 == 3),
                )

        # ---- RMS in two batches: heads 0-3 overlap attention of heads 4-7
        def emit_rms(h_lo, h_hi):
            w = (h_hi - h_lo) * NT
            rstd = comb.tile(
                [128, w], F32, tag=f"rstd{h_lo}", name=f"rstd{h_lo}"
            )
            nc.scalar.activation(
                out=rstd,
                in_=ssq_all[:, h_lo * NT : h_hi * NT],
                func=Act.Sqrt,
                bias=eps_sb[:],
                scale=1.0 / DV,
            )
            nc.vector.reciprocal(out=rstd, in_=rstd)
            for h in range(h_lo, h_hi):
                for tj in range(NT):
                    nc.vector.tensor_scalar_mul(
                        out=outcat_sb[:, tj, h * DV : (h + 1) * DV],
                        in0=oh_all[(h, tj)],
                        scalar1=rstd[:, (h - h_lo) * NT + tj : (h - h_lo) * NT + tj + 1],
                    )


        for h in range(HPG):
            qc = h // 2
            kc_ = 4 + h // 2
            pbase = [(2 * h % 4) * 32, (2 * h % 4) * 32 + 32]
            s_tiles = [
                s_pool.tile([128, T], F32, tag=f"s{e}", name=f"s{e}_{h}")
                for e in range(2)
            ]
            u_tiles = [
                [
                    u_pool.tile(
                        [128, 512], F32, tag=f"u{e}{b}", name=f"u{e}{b}_{h}"
                    )
                    for b in range(2)
                ]
                for e in range(2)
            ]
            if h == 0:
                # s=0 AVs immediately; s=1 becomes the delayed pair
                s0_, es0_, ch0_ = prebaked[0]
                emit_av(h, s0_, 0, es0_[0], ch0_)
                emit_av(h, s0_, 1, es0_[1], ch0_)
                prev = prebaked[1]
                s_start = 2
            else:
                prev = None
                s_start = 0
            for s in range(s_start, NT):
                t0 = 128 * s
                chunks = [(t0, 512), (512, 1024)] if s < 4 else [(t0, 1024)]
                es_s = [
                    es_pool.tile([128, T], F16, tag=f"es{e}", name=f"es{e}_{h}_{s}")
                    for e in range(2)
                ]
                for c0, c1 in chunks:
                    for e in range(2):
                        nc.tensor.matmul(
                            s_tiles[e][:, c0:c1],
                            qkT_sb[:, kc_, t0 : t0 + 128],
                            qTp_sb[:, 2 * h + e, c0:c1],
                            start=True,
                            stop=True,
                        )
                if prev is not None:
                    ps_, pes_, pchunks_ = prev
                    emit_av(h, ps_, 0, pes_[0], pchunks_)
                    emit_av(h, ps_, 1, pes_[1], pchunks_)
                for e in range(2):
                    nc.scalar.activation(
                        out=es_s[e][:, t0:T],
                        in_=s_tiles[e][:, t0:T],
                        func=Act.Exp,
                        scale=1.0 / 32.0,
                    )
                    # causal mask inside the diagonal block: keep t >= s
                    nc.gpsimd.affine_select(
                        out=es_s[e][:, t0 : t0 + 128],
                        in_=es_s[e][:, t0 : t0 + 128],
                        pattern=[[1, 128]],
                        compare_op=AluOp.is_ge,
                        fill=0.0,
                        base=0,
                        channel_multiplier=-1,
                    )
                prev = (s, es_s, chunks)
            ps_, pes_, pchunks_ = prev
            emit_av(h, ps_, 0, pes_[0], pchunks_)
            emit_av(h, ps_, 1, pes_[1], pchunks_)

            # ---- normalize, lambda-combine, accumulate sum-of-squares ----
            for tj in range(NT):
                off = (tj % 4) * 128
                up = [u_tiles[e][tj // 4] for e in range(2)]
                rr_ = [
                    comb.tile([128, 1], F32, tag=f"r{e}", name=f"r{e}_{h}_{tj}")
                    for e in range(2)
                ]
                for e in range(2):
                    nc.vector.reciprocal(
                        out=rr_[e], in_=up[e][:, off + DV : off + DV + 1]
                    )
                t2 = comb.tile([128, DV], F32, tag="t2", name=f"t2_{h}_{tj}")
                nc.vector.tensor_scalar(
                    out=t2,
                    in0=up[1][:, off : off + DV],
                    scalar1=rr_[1][:],
                    scalar2=lam_sb[:],
                    op0=AluOp.mult,
                    op1=AluOp.mult,
                )
                oh = ohp.tile(
                    [128, DV], F32, tag=f"oh{h}_{tj}", name=f"oh_{h}_{tj}"
                )
                nc.vector.scalar_tensor_tensor(
                    out=oh,
                    in0=up[0][:, off : off + DV],
                    scalar=rr_[0][:],
                    in1=t2,
                    op0=AluOp.mult,
                    op1=AluOp.subtract,
                )
                sq = comb.tile([128, DV], F32, tag="sq", name=f"sq_{h}_{tj}")
                nc.vector.scalar_tensor_tensor(
                    out=sq,
                    in0=oh,
                    scalar=1.0,
                    in1=oh,
                    op0=AluOp.mult,
                    op1=AluOp.mult,
                    accum_out=ssq_all[:, h * NT + tj : h * NT + tj + 1],
                )
                oh_all[(h, tj)] = oh
            if h == 3:
                emit_rms(0, 4)

        emit_rms(4, HPG)

    # ---------------- phase 4+5: transpose + output projection ----------------
    with (
        tc.tile_pool(name="tps", bufs=2, space="PSUM") as tps,
        tc.tile_pool(name="pps", bufs=4, space="PSUM") as pps,
        tc.tile_pool(name="yout", bufs=2) as yout,
    ):
        for tt in range(NT):
            nc.sync.dma_start_transpose(
                out=outcatT_sb[:, :, tt * 128 : (tt + 1) * 128],
                in_=outcat_sb[:, tt, :],
            )
        for tt in range(NT):
            yt = yout.tile([128, C], F32, tag="yt", name=f"y{tt}")
            for nh in range(2):
                ps = pps.tile([128, 512], F32, tag="pp", name=f"pp{tt}{nh}")
                for rr in range(4):
                    nc.tensor.matmul(
                        ps,
                        outcatT_sb[:, rr, tt * 128 : (tt + 1) * 128],
                        wp_sb[:, rr, nh * 512 : (nh + 1) * 512],
                        start=(rr == 0),
                        stop=(rr == 3),
                    )
                nc.vector.tensor_copy(out=yt[:, nh * 512 : (nh + 1) * 512], in_=ps)
            nc.sync.dma_start(out=y[tt * 128 : (tt + 1) * 128, :], in_=yt)


def build_nc():
    nc = bass.Bass()
    xT = nc.declare_dram_parameter("xT", [C, T], F16, isOutput=False)
    w_qk = nc.declare_dram_parameter("w_qk", [C, COLS], F16, isOutput=False)
    w_v = nc.declare_dram_parameter("w_v", [C, 512], F16, isOutput=False)
    w_p = nc.declare_dram_parameter("w_p", [512, C], F16, isOutput=False)
    lam = nc.declare_dram_parameter("lam", [128, 1], F32, isOutput=False)
    y = nc.declare_dram_parameter("y", [T, C], F32, isOutput=True)
    with tile.TileContext(nc) as tc:
        with ExitStack() as ctx:
            _emit(ctx, tc, xT, w_qk, w_v, w_p, lam, y)
    return nc


_NC = None


def _get_nc():
    global _NC
    if _NC is None:
        _NC = build_nc()
    return _NC


def make_in_maps(x, w_attn, w_proj, lambda_q1, lambda_q2, lambda_k1, lambda_k2, gamma):
    x = np.asarray(x, np.float32)
    w_attn = np.asarray(w_attn, np.float32)
    w_proj = np.asarray(w_proj, np.float32)
    lam1 = np.exp(np.sum(np.float32(lambda_q1) * np.float32(lambda_k1), dtype=np.float32))
    lam2 = np.exp(np.sum(np.float32(lambda_q2) * np.float32(lambda_k2), dtype=np.float32))
    lam_full = np.float32(lam1 - lam2 + LAMBDA_INIT)
    lam_tile = np.full((128, 1), lam_full, np.float32)
    # fold gamma * (1 - lambda_init) into w_proj rows
    scale = np.tile(np.asarray(gamma, np.float32), H_TOT) * np.float32(1.0 - LAMBDA_INIT)
    w_p_full = (w_proj * scale[:, None]).astype(np.float16)

    in_maps = []
    for core in range(N_CORES):
        b, g = core // G, core % G
        in_maps.append(
            {
                "xT": np.ascontiguousarray(x[b].T.astype(np.float16)),
                "w_qk": np.ascontiguousarray(
                    np.concatenate(
                        [
                            w_attn[:, g * 512 : (g + 1) * 512],
                            w_attn[:, C + g * 512 : C + (g + 1) * 512],
                        ],
                        axis=1,
                    ).astype(np.float16)
                ),
                "w_v": np.ascontiguousarray(
                    w_attn[:, 2 * C + g * 512 : 2 * C + (g + 1) * 512].astype(
                        np.float16
                    )
                ),
                "w_p": np.ascontiguousarray(w_p_full[g * 512 : (g + 1) * 512, :]),
                "lam": lam_tile,
            }
        )
    return in_maps


def assemble(results):
    y = np.empty((B, T, C), np.float32)
    for b in range(B):
        y[b] = results[b * G]["y"] + results[b * G + 1]["y"]
    return y


def kernel(**inputs) -> np.ndarray:
    nc = _get_nc()
    in_maps = make_in_maps(**inputs)
    res = run_bass_kernel_spmd(nc, in_maps, list(range(N_CORES)))
    return assemble(res.results)


# revision 28
# speedup vs baseline: 1.4656x; 1.0079x over previous
"""MultiHeadDiffAttn Trainium2 kernel (v2, 16-bit matmul path).

Sharding: 8 cores = 4-way data parallel over batch x 2-way tensor parallel
over heads (8 v-heads / 16 half-heads per core).  Each core computes its
batch's qkv projection restricted to its head group, differential attention
with per-half-head softmax, head RMS norm, and a partial output projection
(its 512 rows of w_proj).  Host sums the two partial projections per batch.

Measured device behavior this kernel is shaped around:
  - fp32 matmul streams at ~1/4 the 16-bit rate (and fp32 K=32 at 1/8), so
    all matmul operands are fp16 (PSUM accumulation stays fp32).
  - K=32 16-bit matmuls stream at 2 cycles/col; the two half-heads of a
    head sit at different 32-row strips of the PE array, so their S^T
    matmuls are emitted adjacently with explicit tile_position to overlap.
  - ACT exp costs ~0.83ns/col + ~360ns/op fixed, so exp runs once per
    (half-head, s-block) over the whole [128, up-to-1024] PSUM row-block.
  - AV runs transposed (U^T[dv,t] = v_aug^T @ expS^T) so it is
    stream-bound with a tiny reused LDWEIGHTS; a ones-column appended to v
    makes row 64 of U^T the softmax denominator.  The den row is
    reciprocal'd in SBUF before PE-transposing U^T back to [t, dv+1], so
    normalization + lambda-combine + RMS are native per-partition ops.
"""

import math
from contextlib import ExitStack

import numpy as np

import concourse.bass as bass
import concourse.tile as tile
from concourse import masks, mybir
from concourse.bass_utils import run_bass_kernel_spmd

# The deployed walrus rejects instructions carrying more than one sync wait
# ("Too many sync wait commands" in setupSyncWait).  Legalize at the BIR-JSON
# level: for every instruction with >1 wait, hoist the extra waits onto NoOp
# instructions inserted just before it on the same engine (engine streams are
# in-order, so semantics are identical).
_MAX_WAITS = 1


def _legalize_sync_waits(d):
    for f in d.get("functions", []):
        for bb in f.get("blocks", []):
            out = []
            for inst in bb["instructions"]:
                si = inst.get("sync_info")
                waits = (si or {}).get("on_wait") or []
                if len(waits) > _MAX_WAITS:
                    extra = waits[: len(waits) - _MAX_WAITS]
                    keep = waits[len(waits) - _MAX_WAITS :]
                    for j in range(0, len(extra), _MAX_WAITS):
                        nop = {
                            "engine": inst["engine"],
                            "ins": [],
                            "outs": [],
                            "name": f"{inst['name']}-lw{j}",
                            "opcode": "NoOp",
                            "sync_info": {
                                "on_wait": extra[j : j + _MAX_WAITS],
                                "on_update": [],
                            },
                        }
                        if "debug" in inst:
                            nop["debug"] = inst["debug"]
                        out.append(nop)
                    si["on_wait"] = keep
                out.append(inst)
            bb["instructions"] = out
    return d


_orig_to_json_bytes = bass.Bass.to_json_bytes


def _patched_to_json_bytes(self, *a, **kw):
    import json as _json

    raw = _orig_to_json_bytes(self, *a, **kw)
    return _json.dumps(_legalize_sync_waits(_json.loads(raw))).encode()


bass.Bass.to_json_bytes = _patched_to_json_bytes

F32 = mybir.dt.float32
F16 = mybir.dt.float16

B, T, C = 4, 1024, 1024
H_TOT = 16  # total v-heads
HD = 32  # half-head dim
DV = 64  # v-head dim
G = 2  # head groups (tensor parallel)
HPG = H_TOT // G  # 8 v-heads per core
COLS = 1024  # q cols + k cols per group
LAMBDA_INIT = 0.8 - 0.6 * math.exp(-0.3 * (1 - 1))  # 0.2
EPS = 1e-5
N_CORES = 8

NT = T // 128  # 8 t-tiles
NKC = C // 128  # 8 contraction chunks


def _emit(ctx: ExitStack, tc: tile.TileContext, xT, w_qk, w_v, w_p, lam, y):
    nc = tc.nc
    AluOp = mybir.AluOpType
    Act = mybir.ActivationFunctionType

    const = ctx.enter_context(tc.tile_pool(name="const", bufs=1))
    ident = const.tile([128, 128], F16)
    masks.make_identity(nc, ident[:])
    lam_sb = const.tile([128, 1], F32)
    nc.sync.dma_start(out=lam_sb, in_=lam[:])
    eps_sb = const.tile([128, 1], F32)
    nc.vector.memset(eps_sb, EPS)

    big = ctx.enter_context(tc.tile_pool(name="big", bufs=1))
    qkT_sb = big.tile([128, 8, T], F16)  # row-chunks of [COLS, T]
    v_sb = big.tile([128, NT, HPG, 128], F16)  # [s-chunk][head][dv | ones | 0-pad]
    outcat_sb = big.tile([128, NT, HPG * DV], F16)  # [t-chunk][512]
    outcatT_sb = big.tile([128, 4, T], F16)  # row-chunks of [512, T]
    wp_sb = big.tile([128, 4, C], F16)
    # per-half-head q, zero-padded to K=128: data lives at the same 32-row
    # strip as that half-head's k rows inside its qkT chunk, so the S^T
    # matmul can contract over the full 128 partitions at full stream rate
    # (the other half-heads' k rows meet zero q rows).
    qTp_sb = big.tile([128, 2 * HPG, T], F16)

    nc.gpsimd.memset(v_sb[:, :, :, DV + 1 :], 0.0)
    es_pool = ctx.enter_context(tc.tile_pool(name="es", bufs=3))
    nc.gpsimd.memset(qTp_sb, 0.0)

    # ---------------- phase 1+2: qkv projections ----------------
    with (
        tc.tile_pool(name="xw", bufs=1) as xw,
        tc.tile_pool(name="mmps", bufs=4, space="PSUM") as mmps,
    ):
        xT_sb = xw.tile([128, NKC, T], F16)
        wqk_sb = xw.tile([128, NKC, COLS], F16)
        wv_sb = xw.tile([128, NKC, 512], F16)

        xT_r = xT[:].rearrange("(c p) t -> p c t", p=128)
        wqk_r = w_qk[:].rearrange("(c p) m -> p c m", p=128)

        def load_wqk(cc):
            nc.sync.dma_start(
                out=wqk_sb[:, :, cc * 128 : (cc + 1) * 128],
                in_=wqk_r[:, :, cc * 128 : (cc + 1) * 128],
            )

        load_wqk(0)
        for nh in range(2):  # t-halves so first matmuls start early
            nc.sync.dma_start(
                out=xT_sb[:, :, nh * 512 : (nh + 1) * 512],
                in_=xT_r[:, :, nh * 512 : (nh + 1) * 512],
            )
        for cc in range(1, 8):
            load_wqk(cc)
        nc.sync.dma_start(out=wv_sb, in_=w_v[:].rearrange("(c p) m -> p c m", p=128))
        nc.sync.dma_start(
            out=wp_sb, in_=w_p[:].rearrange("(c p) m -> p c m", p=128)
        )

        # qkT[cc-block, :] = w_qk[:, cc-block].T @ x^T
        for cc in range(8):
            for nh in range(2):
                ps = mmps.tile([128, 1024], F32, tag="mmps", name=f"qk{cc}{nh}")[:, 0:512]
                for kc in range(NKC):
                    nc.tensor.matmul(
                        ps,
                        wqk_sb[:, kc, cc * 128 : (cc + 1) * 128],
                        xT_sb[:, kc, nh * 512 : (nh + 1) * 512],
                        start=(kc == 0),
                        stop=(kc == NKC - 1),
                    )
                nc.vector.tensor_copy(
                    out=qkT_sb[:, cc, nh * 512 : (nh + 1) * 512], in_=ps
                )
                if cc < 4:  # q chunk: scatter the 4 half-heads into qTp
                    for j in range(4):
                        hh = cc * 4 + j
                        nc.vector.tensor_copy(
                            out=qTp_sb[
                                j * 32 : (j + 1) * 32,
                                hh,
                                nh * 512 : (nh + 1) * 512,
                            ],
                            in_=qkT_sb[
                                j * 32 : (j + 1) * 32,
                                cc,
                                nh * 512 : (nh + 1) * 512,
                            ],
                        )

        # prebake head 0, s=0..1: S/exp/mask run during the v projection
        prebaked = []
        for s in range(2):
            t0 = 128 * s
            chunks = [(t0, 512), (512, 1024)]
            es_s = [
                es_pool.tile([128, T], F16, tag=f"es{e}", name=f"esp{e}_{s}")
                for e in range(2)
            ]
            for e in range(2):
                sps = mmps.tile([128, 1024], F32, tag="mmps", name=f"sp{e}_{s}")
                for c0, c1 in chunks:
                    nc.tensor.matmul(
                        sps[:, c0:c1],
                        qkT_sb[:, 4, t0 : t0 + 128],
                        qTp_sb[:, e, c0:c1],
                        start=True,
                        stop=True,
                    )
                nc.scalar.activation(
                    out=es_s[e][:, t0:T],
                    in_=sps[:, t0:T],
                    func=Act.Exp,
                    scale=1.0 / 32.0,
                )
                nc.gpsimd.affine_select(
                    out=es_s[e][:, t0 : t0 + 128],
                    in_=es_s[e][:, t0 : t0 + 128],
                    pattern=[[1, 128]],
                    compare_op=AluOp.is_ge,
                    fill=0.0,
                    base=0,
                    channel_multiplier=-1,
                )
            prebaked.append((s, es_s, chunks))

        # v[t-block, :] = x @ w_v ; scatter heads into v_sb, slot 64 = ones
        for tt in range(NT):
            ps = mmps.tile([128, 1024], F32, tag="mmps", name=f"v{tt}")[:, 0:512]
            for kc in range(NKC):
                nc.tensor.matmul(
                    ps,
                    xT_sb[:, kc, tt * 128 : (tt + 1) * 128],
                    wv_sb[:, kc, :],
                    start=(kc == 0),
                    stop=(kc == NKC - 1),
                )
            nc.vector.tensor_copy(
                out=v_sb[:, tt, :, 0:DV],
                in_=ps[:].rearrange("p (h d) -> p h d", h=HPG),
            )
            nc.vector.memset(v_sb[:, tt, :, DV : DV + 1], 1.0)

    # ---------------- phase 3: differential attention ----------------
    with (
        tc.tile_pool(name="us", bufs=2) as us_pool,
        tc.tile_pool(name="sps", bufs=1, space="PSUM") as s_pool,
        tc.tile_pool(name="ups", bufs=1, space="PSUM") as u_pool,
        tc.tile_pool(name="comb", bufs=3) as comb,
        tc.tile_pool(name="ohp", bufs=1, space="SBUF") as ohp,
    ):
        oh_all = {}
        ssq_all = ohp.tile([128, HPG * NT], F32, tag="ssqall", name="ssqall")

        def emit_av(h, s, e, es_s, chunks):
            # U[t-block, dv|den] += expS^T[s-block, t-block].T @ v_aug[s-block]
            for tj in range(s, NT):
                off = (tj % 4) * 128
                nc.tensor.matmul(
                    u_tiles[e][tj // 4][:, off : off + DV + 1],
                    es_s[:, tj * 128 : (tj + 1) * 128],
                    v_sb[:, s, h, 0 : DV + 1],
                    start=(s == 0 and tj % 4 == 0),
                    stop=(s == tj and tj % 4 == 3),
                )

        # ---- RMS in two batches: heads 0-3 overlap attention of heads 4-7
        def emit_rms(h_lo, h_hi):
            w = (h_hi - h_lo) * NT
            rstd = comb.tile(
                [128, w], F32, tag=f"rstd{h_lo}", name=f"rstd{h_lo}"
            )
            nc.scalar.activation(
                out=rstd,
                in_=ssq_all[:, h_lo * NT : h_hi * NT],
                func=Act.Sqrt,
                bias=eps_sb[:],
                scale=1.0 / DV,
            )
            nc.vector.reciprocal(out=rstd, in_=rstd)
            for tj in range(NT):  # tj-outer so outcat tiles complete in order
                for h in range(h_lo, h_hi):
                    nc.vector.tensor_scalar_mul(
                        out=outcat_sb[:, tj, h * DV : (h + 1) * DV],
                        in0=oh_all[(h, tj)],
                        scalar1=rstd[:, (h - h_lo) * NT + tj : (h - h_lo) * NT + tj + 1],
                    )


        for h in range(HPG):
            qc = h // 2
            kc_ = 4 + h // 2
            pbase = [(2 * h % 4) * 32, (2 * h % 4) * 32 + 32]
            s_tiles = [
                s_pool.tile([128, T], F32, tag=f"s{e}", name=f"s{e}_{h}")
                for e in range(2)
            ]
            u_tiles = [
                [
                    u_pool.tile(
                        [128, 512], F32, tag=f"u{e}{b}", name=f"u{e}{b}_{h}"
                    )
                    for b in range(2)
                ]
                for e in range(2)
            ]
            if h == 0:
                # s=0 AVs immediately; s=1 becomes the delayed pair
                s0_, es0_, ch0_ = prebaked[0]
                emit_av(h, s0_, 0, es0_[0], ch0_)
                emit_av(h, s0_, 1, es0_[1], ch0_)
                prev = prebaked[1]
                s_start = 2
            else:
                prev = None
                s_start = 0
            for s in range(s_start, NT):
                t0 = 128 * s
                chunks = [(t0, 512), (512, 1024)] if s < 4 else [(t0, 1024)]
                es_s = [
                    es_pool.tile([128, T], F16, tag=f"es{e}", name=f"es{e}_{h}_{s}")
                    for e in range(2)
                ]
                for c0, c1 in chunks:
                    for e in range(2):
                        nc.tensor.matmul(
                            s_tiles[e][:, c0:c1],
                            qkT_sb[:, kc_, t0 : t0 + 128],
                            qTp_sb[:, 2 * h + e, c0:c1],
                            start=True,
                            stop=True,
                        )
                if prev is not None:
                    ps_, pes_, pchunks_ = prev
                    emit_av(h, ps_, 0, pes_[0], pchunks_)
                    emit_av(h, ps_, 1, pes_[1], pchunks_)
                for e in range(2):
                    nc.scalar.activation(
                        out=es_s[e][:, t0:T],
                        in_=s_tiles[e][:, t0:T],
                        func=Act.Exp,
                        scale=1.0 / 32.0,
                    )
                    # causal mask inside the diagonal block: keep t >= s
                    nc.gpsimd.affine_select(
                        out=es_s[e][:, t0 : t0 + 128],
                        in_=es_s[e][:, t0 : t0 + 128],
                        pattern=[[1, 128]],
                        compare_op=AluOp.is_ge,
                        fill=0.0,
                        base=0,
                        channel_multiplier=-1,
                    )
                prev = (s, es_s, chunks)
            ps_, pes_, pchunks_ = prev
            emit_av(h, ps_, 0, pes_[0], pchunks_)
            emit_av(h, ps_, 1, pes_[1], pchunks_)

            # ---- normalize, lambda-combine, accumulate sum-of-squares ----
            for tj in range(NT):
                off = (tj % 4) * 128
                up = [u_tiles[e][tj // 4] for e in range(2)]
                rr_ = [
                    comb.tile([128, 1], F32, tag=f"r{e}", name=f"r{e}_{h}_{tj}")
                    for e in range(2)
                ]
                for e in range(2):
                    nc.vector.reciprocal(
                        out=rr_[e], in_=up[e][:, off + DV : off + DV + 1]
                    )
                t2 = comb.tile([128, DV], F32, tag="t2", name=f"t2_{h}_{tj}")
                nc.vector.tensor_scalar(
                    out=t2,
                    in0=up[1][:, off : off + DV],
                    scalar1=rr_[1][:],
                    scalar2=lam_sb[:],
                    op0=AluOp.mult,
                    op1=AluOp.mult,
                )
                oh = ohp.tile(
                    [128, DV], F32, tag=f"oh{h}_{tj}", name=f"oh_{h}_{tj}"
                )
                nc.vector.scalar_tensor_tensor(
                    out=oh,
                    in0=up[0][:, off : off + DV],
                    scalar=rr_[0][:],
                    in1=t2,
                    op0=AluOp.mult,
                    op1=AluOp.subtract,
                )
                sq = comb.tile([128, DV], F32, tag="sq", name=f"sq_{h}_{tj}")
                nc.vector.scalar_tensor_tensor(
                    out=sq,
                    in0=oh,
                    scalar=1.0,
                    in1=oh,
                    op0=AluOp.mult,
                    op1=AluOp.mult,
                    accum_out=ssq_all[:, h * NT + tj : h * NT + tj + 1],
                )
                oh_all[(h, tj)] = oh
            if h == 3:
                emit_rms(0, 4)

        emit_rms(4, HPG)

    # ---------------- phase 4+5: transpose + output projection ----------------
    with (
        tc.tile_pool(name="tps", bufs=2, space="PSUM") as tps,
        tc.tile_pool(name="pps", bufs=4, space="PSUM") as pps,
        tc.tile_pool(name="yout", bufs=2) as yout,
    ):
        for tt in range(NT):
            nc.sync.dma_start_transpose(
                out=outcatT_sb[:, :, tt * 128 : (tt + 1) * 128],
                in_=outcat_sb[:, tt, :],
            )
        for tt in range(NT):
            yt = yout.tile([128, C], F32, tag="yt", name=f"y{tt}")
            for nh in range(2):
                ps = pps.tile([128, 512], F32, tag="pp", name=f"pp{tt}{nh}")
                for rr in range(4):
                    nc.tensor.matmul(
                        ps,
                        outcatT_sb[:, rr, tt * 128 : (tt + 1) * 128],
                        wp_sb[:, rr, nh * 512 : (nh + 1) * 512],
                        start=(rr == 0),
                        stop=(rr == 3),
                    )
                nc.vector.tensor_copy(out=yt[:, nh * 512 : (nh + 1) * 512], in_=ps)
            nc.sync.dma_start(out=y[tt * 128 : (tt + 1) * 128, :], in_=yt)


def build_nc():
    nc = bass.Bass()
    xT = nc.declare_dram_parameter("xT", [C, T], F16, isOutput=False)
    w_qk = nc.declare_dram_parameter("w_qk", [C, COLS], F16, isOutput=False)
    w_v = nc.declare_dram_parameter("w_v", [C, 512], F16, isOutput=False)
    w_p = nc.declare_dram_parameter("w_p", [512, C], F16, isOutput=False)
    lam = nc.declare_dram_parameter("lam", [128, 1], F32, isOutput=False)
    y = nc.declare_dram_parameter("y", [T, C], F32, isOutput=True)
    with tile.TileContext(nc) as tc:
        with ExitStack() as ctx:
            _emit(ctx, tc, xT, w_qk, w_v, w_p, lam, y)
    return nc


_NC = None


def _get_nc():
    global _NC
    if _NC is None:
        _NC = build_nc()
    return _NC


def make_in_maps(x, w_attn, w_proj, lambda_q1, lambda_q2, lambda_k1, lambda_k2, gamma):
    x = np.asarray(x, np.float32)
    w_attn = np.asarray(w_attn, np.float32)
    w_proj = np.asarray(w_proj, np.float32)
    lam1 = np.exp(np.sum(np.float32(lambda_q1) * np.float32(lambda_k1), dtype=np.float32))
    lam2 = np.exp(np.sum(np.float32(lambda_q2) * np.float32(lambda_k2), dtype=np.float32))
    lam_full = np.float32(lam1 - lam2 + LAMBDA_INIT)
    lam_tile = np.full((128, 1), lam_full, np.float32)
    # fold gamma * (1 - lambda_init) into w_proj rows
    scale = np.tile(np.asarray(gamma, np.float32), H_TOT) * np.float32(1.0 - LAMBDA_INIT)
    w_p_full = (w_proj * scale[:, None]).astype(np.float16)

    in_maps = []
    for core in range(N_CORES):
        b, g = core // G, core % G
        in_maps.append(
            {
                "xT": np.ascontiguousarray(x[b].T.astype(np.float16)),
                "w_qk": np.ascontiguousarray(
                    np.concatenate(
                        [
                            w_attn[:, g * 512 : (g + 1) * 512],
                            w_attn[:, C + g * 512 : C + (g + 1) * 512],
                        ],
                        axis=1,
                    ).astype(np.float16)
                ),
                "w_v": np.ascontiguousarray(
                    w_attn[:, 2 * C + g * 512 : 2 * C + (g + 1) * 512].astype(
                        np.float16
                    )
                ),
                "w_p": np.ascontiguousarray(w_p_full[g * 512 : (g + 1) * 512, :]),
                "lam": lam_tile,
            }
        )
    return in_maps


def assemble(results):
    y = np.empty((B, T, C), np.float32)
    for b in range(B):
        y[b] = results[b * G]["y"] + results[b * G + 1]["y"]
    return y


def kernel(**inputs) -> np.ndarray:
    nc = _get_nc()
    in_maps = make_in_maps(**inputs)
    res = run_bass_kernel_spmd(nc, in_maps, list(range(N_CORES)))
    return assemble(res.results)
